# revision 28
# baseline (speedup 1.0000x reference)
"""EHM (SMPLX body + FLAME head + MANO hands) Bass kernel for 8 TRN2 NeuronCores.

Sharding: VERTEX sharding — model weights (shapedirs/posedirs/regressors/lbs
weights, ~130MB) dominate HBM traffic, so each core owns 1/8 of the SMPLX
vertices (plus the FLAME/MANO vertices its SMPLX rows stitch in) and computes
ALL B=128 batch elements for its shard.  The only cross-core dependency is the
joint regression J = J_regressor @ v_shaped -> one [76, 384] AllReduce of
partial joint sums.  FK (92 joints) is replicated on every core on the vector
engine with batch on partitions (B == 128 == n_partitions).

Per-vertex data layout: [vertex(partition<=128), (c, b)] with c-major free dim
(col = c*128 + b).  Batch-staged data (poses, FK, A matrices): [b(part), free].
"""

import sys

sys.path.insert(0, "/opt/trn_rl_repo")

from contextlib import ExitStack

import numpy as np
import ml_dtypes

BF16NP = ml_dtypes.bfloat16

import concourse.bass as bass
import concourse.bacc as bacc
import concourse.tile as tile
import concourse.mybir as mybir
from concourse.bass_utils import run_bass_kernel_spmd

F32 = mybir.dt.float32
BF16 = mybir.dt.bfloat16
AF = mybir.ActivationFunctionType
ALU = mybir.AluOpType

# ---------------------------------------------------------------- constants
B = 128
VS, VF, VM = 10475, 5023, 778
NL = 350
NCORES = 8

SMPLX_PARENTS = np.array([-1,0,0,0,1,2,3,4,5,6,7,8,9,9,9,12,13,14,16,17,18,19,
                          15,15,15,20,25,26,20,28,29,20,31,32,20,34,35,20,37,38,
                          21,40,41,21,43,44,21,46,47,21,49,50,21,52,53])
FLAME_PARENTS = np.array([-1,0,1,1,1])
MANO_PARENTS = np.array([-1,0,1,2,0,4,5,0,7,8,0,10,11,0,13,14])

N_PLAIN, N_HEAD, N_HL, N_HR = 768, 384, 128, 128
ROWS = N_PLAIN + N_HEAD + N_HL + N_HR        # 1408
NCH = ROWS // 128                            # 11
CH_PLAIN = set(range(0, 6))
CH_HEAD0 = 6                                 # chunks 6,7,8 head; 9 L; 10 R
CH_HL, CH_HR = 9, 10

NFE_CH = 5
NMJ_CH = 3
PD_S_K = 189
PD_F_K = 27
PD_M_K = 135

NJ_ALL = 92
OFF_S, OFF_F, OFF_L, OFF_R = 0, 55, 60, 76
NROT = 55
ROT_S0, ROT_F0, ROT_L0, ROT_R0 = 0, 22, 25, 40

BF16_INPUTS = {"w_s", "wre_f", "w_m", "ancT_s", "ancT_f", "ancT_m",
               "sd_s", "pd_s_a", "pd_s_b", "jr_s", "sd_f", "jr_f", "pd_f",
               "sd_m", "pd_m_a", "pd_m_b", "sd_mj", "jreg_m",
               "betaT_s", "betaT_f", "betam"}


def _fk_forest():
    par = np.empty(NJ_ALL, np.int64)
    par[OFF_S:OFF_S + 55] = SMPLX_PARENTS
    par[OFF_F:OFF_F + 5] = np.where(FLAME_PARENTS < 0, -1, FLAME_PARENTS + OFF_F)
    par[OFF_L:OFF_L + 16] = np.where(MANO_PARENTS < 0, -1, MANO_PARENTS + OFF_L)
    par[OFF_R:OFF_R + 16] = np.where(MANO_PARENTS < 0, -1, MANO_PARENTS + OFF_R)
    return par


def _fk_levels(par):
    depth = np.zeros(NJ_ALL, np.int64)
    for j in range(NJ_ALL):
        if par[j] >= 0:
            depth[j] = depth[par[j]] + 1
    levels = []
    for d in range(1, int(depth.max()) + 1):
        js = np.nonzero(depth == d)[0]
        runs, i = [], 0
        while i < len(js):
            j0, p0 = int(js[i]), int(par[js[i]])
            if i + 1 < len(js):
                ds = int(js[i + 1]) - j0
                ps = int(par[js[i + 1]]) - p0
            else:
                ds, ps = 1, 0
            n = 1
            while (i + n < len(js) and int(js[i + n]) == j0 + n * ds
                   and int(par[js[i + n]]) == p0 + n * ps):
                n += 1
            if n == 1:
                ds, ps = 1, 0
            runs.append((j0, ds, n, p0, ps))
            i += n
        levels.append(runs)
    return levels


# ================================================================ host prep

def _split_sizes(total, parts):
    q, r = divmod(total, parts)
    return [q + (1 if i < r else 0) for i in range(parts)]


def _pad_ids(ids, n):
    out = np.full(n, -1, np.int64)
    out[:len(ids)] = ids
    return out


def _host_prep(inp):
    f32 = np.float32
    s2f = np.asarray(inp["smplx2flame_ind"])
    head_ix = np.asarray(inp["head_index"])
    s2l = np.asarray(inp["smplx2mano_left"])
    s2r = np.asarray(inp["smplx2mano_right"])

    head_sv = s2f[head_ix]
    special = np.zeros(VS, bool)
    special[head_sv] = True
    special[s2l] = True
    special[s2r] = True
    plain_sv = np.nonzero(~special)[0]

    pl_sp = np.cumsum([0] + _split_sizes(len(plain_sv), NCORES))
    hd_sp = np.cumsum([0] + _split_sizes(len(head_ix), NCORES))
    hl_sp = np.cumsum([0] + _split_sizes(VM, NCORES))
    fe_sp = np.cumsum([0] + _split_sizes(VF, NCORES))
    mj_sp = np.cumsum([0] + _split_sizes(VM * 3, NCORES))

    sd_s_np = np.asarray(inp["smplx_shapedirs"], f32)
    pd_s_np = np.asarray(inp["smplx_posedirs"], f32)
    jr_s_np = np.asarray(inp["smplx_J_regressor"], f32)
    w_s_np = np.asarray(inp["smplx_lbs_weights"], f32)
    tmpl_s = np.asarray(inp["smplx_v_template"], f32)
    sd_f_np = np.asarray(inp["flame_shapedirs"], f32)
    pd_f_np = np.asarray(inp["flame_posedirs"], f32)
    jr_f_np = np.asarray(inp["flame_J_regressor"], f32)
    w_f_np = np.asarray(inp["flame_lbs_weights"], f32)
    tmpl_f = np.asarray(inp["flame_v_template"], f32)
    re_np = np.asarray(inp["r_eyelid"], f32)
    le_np = np.asarray(inp["l_eyelid"], f32)
    sd_m_np = np.asarray(inp["mano_shapedirs"], f32)
    pd_m_np = np.asarray(inp["mano_posedirs"], f32)
    jr_m_np = np.asarray(inp["mano_J_regressor"], f32)
    w_m_np = np.asarray(inp["mano_lbs_weights"], f32)
    tmpl_m = np.asarray(inp["mano_v_template"], f32)

    aa = np.concatenate([
        np.asarray(inp["global_pose"], f32).reshape(B, 3),
        np.asarray(inp["body_pose"], f32).reshape(B, 63),
        np.asarray(inp["jaw_params"], f32).reshape(B, 3),
        np.asarray(inp["eye_pose"], f32).reshape(B, 6),
        np.asarray(inp["left_hand_pose"], f32).reshape(B, 45),
        np.asarray(inp["right_hand_pose"], f32).reshape(B, 45),
    ], axis=1)

    ep = np.asarray(inp["eyelid_params"], f32)
    aux = np.concatenate([
        np.asarray(inp["head_scale"], f32)[:, None],
        np.asarray(inp["left_hand_scale"], f32)[:, None],
        np.asarray(inp["right_hand_scale"], f32)[:, None],
        ep[:, 0:1], ep[:, 1:2],
        np.asarray(inp["head_pos_offset"], f32),
        np.asarray(inp["left_hand_pos_offset"], f32),
        np.asarray(inp["right_hand_pos_offset"], f32),
    ], axis=1)                                               # [128, 14]

    def beta_T(second):
        b = np.concatenate([np.asarray(inp["shape_params"], f32), second], 1)
        bt = np.zeros((384, B), f32)
        bt[:NL] = b.T
        bt[NL] = 1.0
        return bt.reshape(3, 128, B)

    betaT_s = beta_T(np.asarray(inp["body_exp"], f32))
    betaT_f = beta_T(np.asarray(inp["flame_exp"], f32))

    joff = np.asarray(inp["joints_offset"], f32)
    joffT = np.ascontiguousarray(joff.transpose(1, 2, 0)).reshape(55, 384)

    def mrel_T(par, nj):
        m = np.eye(nj, dtype=f32)
        for j in range(1, nj):
            if par[j] >= 0:
                m[j, par[j]] = -1.0
        return np.ascontiguousarray(m.T)

    betam = np.zeros((11, 1), f32)
    betam[:10, 0] = np.asarray(inp["mano_betas"], f32)[0]
    betam[10, 0] = 1.0

    def anc_T(par, nj):
        m = np.zeros((nj, nj), f32)
        for j in range(nj):
            a = j
            while a >= 0:
                m[j, a] = 1.0
                a = par[a]
        return np.ascontiguousarray(m.T)

    fpar = _fk_forest()
    rep = dict(aa=aa, aux=aux, betaT_s=betaT_s, betaT_f=betaT_f, joffT=joffT,
               mrelT_all=mrel_T(fpar, NJ_ALL), betam=betam,
               ancT_s=anc_T(SMPLX_PARENTS, 55), ancT_f=anc_T(FLAME_PARENTS, 5),
               ancT_m=anc_T(MANO_PARENTS, 16),
               ident=np.eye(128, dtype=f32))

    sd_m_flat = sd_m_np.reshape(VM * 3, 10)
    tmpl_m_flat = tmpl_m.reshape(VM * 3)

    in_maps = []
    vid_all = np.full((NCORES, ROWS), -1, np.int64)

    for c in range(NCORES):
        p_ids = plain_sv[pl_sp[c]:pl_sp[c + 1]]
        h_pos = np.arange(hd_sp[c], hd_sp[c + 1])
        h_sv, h_fv = head_sv[h_pos], head_ix[h_pos]
        l_pos = np.arange(hl_sp[c], hl_sp[c + 1])
        r_pos = l_pos                                         # same split for R
        l_sv, r_sv = s2l[l_pos], s2r[r_pos]

        vid = np.full(ROWS, -1, np.int64)
        vid[:len(p_ids)] = p_ids
        vid[N_PLAIN:N_PLAIN + len(h_sv)] = h_sv
        vid[N_PLAIN + N_HEAD:N_PLAIN + N_HEAD + len(l_sv)] = l_sv
        vid[N_PLAIN + N_HEAD + N_HL:N_PLAIN + N_HEAD + N_HL + len(r_sv)] = r_sv
        vid_all[c] = vid
        vok = vid >= 0
        vc = np.where(vok, vid, 0)

        # smplx shapedirs slab [NCH, 128(p=l), (c, lk, v)]
        sdp = np.zeros((ROWS, 3, 384), f32)
        sdp[:, :, :NL] = np.where(vok[:, None, None], sd_s_np[vc], 0.0)
        sdp[:, :, NL] = np.where(vok[:, None], tmpl_s[vc], 0.0)
        slab = sdp.reshape(NCH, 128, 3, 3, 128).transpose(0, 4, 2, 3, 1)
        sd_s = np.ascontiguousarray(slab).reshape(NCH, 128, 1152)

        colv = vc[:, None] * 3 + np.arange(3)[None, :]
        pdv = pd_s_np[:PD_S_K][:, colv]
        pdv = np.where(vok[None, :, None], pdv, 0.0)
        pdv = pdv.reshape(PD_S_K, NCH, 128, 3).transpose(1, 0, 3, 2)
        pd_s_a = np.ascontiguousarray(pdv[:, :128]).reshape(NCH, 128, 384)
        pd_s_b = np.ascontiguousarray(pdv[:, 128:]).reshape(NCH, PD_S_K - 128, 384)

        jr_s = np.ascontiguousarray(
            np.where(vok[:, None], jr_s_np[:, vc].T, 0.0).reshape(NCH, 128, 55))
        w_s = np.ascontiguousarray(
            np.where(vok[:, None], w_s_np[vc], 0.0)
            .reshape(NCH, 128, 55).transpose(0, 2, 1))

        # flame: 5 even + 3 gathered chunks
        fe = _pad_ids(np.arange(fe_sp[c], fe_sp[c + 1]), NFE_CH * 128)
        fg = _pad_ids(h_fv, N_HEAD)
        f_rows = np.concatenate([fe, fg])
        fok = f_rows >= 0
        fc = np.where(fok, f_rows, 0)
        sdfp = np.zeros((len(f_rows), 3, 384), f32)
        sdfp[:, :, :NL] = np.where(fok[:, None, None], sd_f_np[fc], 0.0)
        sdfp[:, :, NL] = np.where(fok[:, None], tmpl_f[fc], 0.0)
        slab = sdfp.reshape(-1, 128, 3, 3, 128).transpose(0, 4, 2, 3, 1)
        sd_f = np.ascontiguousarray(slab).reshape(-1, 128, 1152)

        jr_f = np.ascontiguousarray(
            np.where(fok[:NFE_CH * 128, None], jr_f_np[:, fc[:NFE_CH * 128]].T, 0.0)
            .reshape(NFE_CH, 128, 5))

        fgc, fgok = fc[NFE_CH * 128:], fok[NFE_CH * 128:]
        colf = fgc[:, None] * 3 + np.arange(3)[None, :]
        pdfv = pd_f_np[9:36][:, colf]
        pdfv = np.where(fgok[None, :, None], pdfv, 0.0)
        pdfv = pdfv.reshape(PD_F_K, 3, 128, 3).transpose(1, 0, 3, 2)
        pd_f = np.ascontiguousarray(pdfv).reshape(3, PD_F_K, 384)

        wre = np.zeros((3, 11, 128), f32)
        for k in range(3):
            rows, ok = fgc[k * 128:(k + 1) * 128], fgok[k * 128:(k + 1) * 128]
            wre[k, :5] = np.where(ok[None, :], w_f_np[rows].T, 0.0)
            wre[k, 5:8] = np.where(ok[None, :], re_np[rows].T, 0.0)
            wre[k, 8:11] = np.where(ok[None, :], le_np[rows].T, 0.0)

        # mano hands + J shard
        m_rows = np.stack([_pad_ids(l_pos, 128), _pad_ids(r_pos, 128)])
        mok = m_rows >= 0
        mc = np.where(mok, m_rows, 0)
        sd_m = np.zeros((2, 11, 384), f32)
        pd_m_a = np.zeros((2, 128, 384), f32)
        pd_m_b = np.zeros((2, PD_M_K - 128, 384), f32)
        w_m = np.zeros((2, 16, 128), f32)
        for h in range(2):
            sdm = np.where(mok[h][:, None, None], sd_m_np[mc[h]], 0.0)
            sd_m[h, :10] = sdm.transpose(2, 1, 0).reshape(10, 384)
            sd_m[h, 10] = np.where(mok[h][:, None], tmpl_m[mc[h]], 0.0).T.reshape(384)
            colm = mc[h][:, None] * 3 + np.arange(3)[None, :]
            pdm = pd_m_np[:, colm]
            pdm = np.where(mok[h][None, :, None], pdm, 0.0).transpose(0, 2, 1)
            pd_m_a[h] = pdm[:128].reshape(128, 384)
            pd_m_b[h] = pdm[128:].reshape(PD_M_K - 128, 384)
            w_m[h] = np.where(mok[h][None, :], w_m_np[mc[h]].T, 0.0)

        mj = _pad_ids(np.arange(mj_sp[c], mj_sp[c + 1]), NMJ_CH * 128)
        mjok = mj >= 0
        mjc = np.where(mjok, mj, 0)
        sd_mj = np.concatenate(
            [np.where(mjok[:, None], sd_m_flat[mjc], 0.0),
             np.where(mjok, tmpl_m_flat[mjc], 0.0)[:, None]], 1)
        sd_mj = np.ascontiguousarray(
            sd_mj.reshape(NMJ_CH, 128, 11).transpose(0, 2, 1))
        jreg_m = np.zeros((NMJ_CH * 128, 48), f32)
        vv, cc3 = mjc // 3, mjc % 3
        jj = np.arange(16)
        jreg_m[np.arange(NMJ_CH * 128)[:, None], jj[None, :] * 3 + cc3[:, None]] = \
            np.where(mjok[:, None], jr_m_np[:, vv].T, 0.0)
        jreg_m = jreg_m.reshape(NMJ_CH, 128, 48)

        m = dict(rep)
        m.update(sd_s=sd_s, pd_s_a=pd_s_a, pd_s_b=pd_s_b, jr_s=jr_s, w_s=w_s,
                 sd_f=sd_f, jr_f=jr_f, pd_f=pd_f, wre_f=wre,
                 sd_m=sd_m, pd_m_a=pd_m_a, pd_m_b=pd_m_b, w_m=w_m,
                 sd_mj=sd_mj, jreg_m=jreg_m)
        out = {}
        for k, v in m.items():
            if k in BF16_INPUTS:
                out[k] = np.ascontiguousarray(v.astype(BF16NP))
            else:
                out[k] = np.ascontiguousarray(v, f32)
        in_maps.append(out)

    return in_maps, vid_all


# ================================================================ device IR

def _build_nc():
    nc = bacc.Bacc("TRN2", target_bir_lowering=False, debug=False,
                   num_devices=NCORES)
    di = {}

    def din(name, shape):
        dt = BF16 if name in BF16_INPUTS else F32
        di[name] = nc.dram_tensor(name, list(shape), dt, kind="ExternalInput").ap()

    din("aa", (B, 165)); din("aux", (B, 14))
    din("betaT_s", (3, 128, 128)); din("betaT_f", (3, 128, 128))
    din("joffT", (55, 384))
    din("mrelT_all", (92, 92))
    din("ancT_s", (55, 55)); din("ancT_f", (5, 5)); din("ancT_m", (16, 16))
    din("betam", (11, 1)); din("ident", (128, 128))
    din("sd_s", (NCH, 128, 1152)); din("pd_s_a", (NCH, 128, 384))
    din("pd_s_b", (NCH, PD_S_K - 128, 384))
    din("jr_s", (NCH, 128, 55)); din("w_s", (NCH, 55, 128))
    din("sd_f", (8, 128, 1152)); din("jr_f", (NFE_CH, 128, 5))
    din("pd_f", (3, PD_F_K, 384)); din("wre_f", (3, 11, 128))
    din("sd_m", (2, 11, 384)); din("pd_m_a", (2, 128, 384))
    din("pd_m_b", (2, PD_M_K - 128, 384)); din("w_m", (2, 16, 128))
    din("sd_mj", (NMJ_CH, 11, 128)); din("jreg_m", (NMJ_CH, 128, 48))

    out_d = nc.dram_tensor("out", [ROWS, 384], F32, kind="ExternalOutput").ap()
    dbg_d = None
    if DEBUG:
        dbg_d = nc.dram_tensor("dbg", [128, 4096], F32, kind="ExternalOutput").ap()

    with tile.TileContext(nc) as tc:
        _emit(nc, tc, di, out_d, dbg_d)
    nc.compile()
    return nc


def _emit(nc, tc, di, out_d, dbg_d=None):
    levels = _fk_levels(_fk_forest())
    es = ExitStack()
    persist = es.enter_context(tc.tile_pool(name="persist", bufs=1))
    slabs = es.enter_context(tc.tile_pool(name="slabs", bufs=3))
    acc_cm = tc.tile_pool(name="acc", bufs=4, space="PSUM")
    acc = acc_cm.__enter__()
    acct_cm = tc.tile_pool(name="acct", bufs=2, space="PSUM")
    acct = acct_cm.__enter__()
    jpool_cm = tc.tile_pool(name="jpool", bufs=1, space="PSUM")
    jpool = jpool_cm.__enter__()
    dram = es.enter_context(tc.tile_pool(name="dram", bufs=1, space="DRAM"))

    V, S, G, T, DMA = nc.vector, nc.scalar, nc.gpsimd, nc.tensor, nc.sync

    def ptile(shape, name):
        return persist.tile(list(shape), F32, tag=name, name=name)

    # ---------------- constants / staged inputs --------------------------

    aa = ptile((B, 165), "aa"); DMA.dma_start(aa[:], di["aa"][:])
    aux = ptile((B, 14), "aux"); DMA.dma_start(aux[:], di["aux"][:])

    betaT_s = persist.tile([128, 384], BF16, tag="betaT_s", name="betaT_s")
    betaT_f = persist.tile([128, 384], BF16, tag="betaT_f", name="betaT_f")
    for lk in range(3):
        DMA.dma_start(betaT_s[:, lk * 128:(lk + 1) * 128], di["betaT_s"][lk])
    betam = persist.tile([11, 1], BF16, tag="betam", name="betam"); DMA.dma_start(betam[:], di["betam"][:])

    # preloaded small per-chunk tensors (one DMA each, persist in SBUF)
    jr_all = persist.tile([128, NCH * 55], BF16, tag="jr_all", name="jr_all")
    DMA.dma_start(jr_all[:].rearrange("p (n k) -> p n k", k=55),
                  di["jr_s"][:].rearrange("n p k -> p n k"))
    for lk in range(3):
        DMA.dma_start(betaT_f[:, lk * 128:(lk + 1) * 128], di["betaT_f"][lk])
    ident = ptile((128, 128), "ident")
    DMA.dma_start(ident[:], di["ident"][:])
    joffT = ptile((55, 384), "joffT"); DMA.dma_start(joffT[:], di["joffT"][:])
    mrelT_all = ptile((92, 92), "mrelT_all"); DMA.dma_start(mrelT_all[:], di["mrelT_all"][:])
    jrf_all = persist.tile([128, NFE_CH * 5], BF16, tag="jrf_all", name="jrf_all")
    DMA.dma_start(jrf_all[:].rearrange("p (n k) -> p n k", k=5),
                  di["jr_f"][:].rearrange("n p k -> p n k"))
    jrm_all = persist.tile([128, NMJ_CH * 48], BF16, tag="jrm_all", name="jrm_all")
    DMA.dma_start(jrm_all[:].rearrange("p (n k) -> p n k", k=48),
                  di["jreg_m"][:].rearrange("n p k -> p n k"))
    sdmj_all = persist.tile([11, NMJ_CH * 128], BF16, tag="sdmj_all", name="sdmj_all")
    DMA.dma_start(sdmj_all[:].rearrange("p (n k) -> p n k", k=128),
                  di["sd_mj"][:].rearrange("n p k -> p n k"))
    w_all = persist.tile([55, NCH * 128], BF16, tag="w_all", name="w_all")
    DMA.dma_start(w_all[:].rearrange("p (n k) -> p n k", k=128),
                  di["w_s"][:].rearrange("n p k -> p n k"))
    wre_all = persist.tile([11, 3 * 128], BF16, tag="wre_all", name="wre_all")
    DMA.dma_start(wre_all[:].rearrange("p (n k) -> p n k", k=128),
                  di["wre_f"][:].rearrange("n p k -> p n k"))
    wm_all = persist.tile([16, 2 * 128], BF16, tag="wm_all", name="wm_all")
    DMA.dma_start(wm_all[:].rearrange("p (n k) -> p n k", k=128),
                  di["w_m"][:].rearrange("n p k -> p n k"))
    sdm_all = persist.tile([11, 2 * 384], BF16, tag="sdm_all", name="sdm_all")
    DMA.dma_start(sdm_all[:].rearrange("p (n k) -> p n k", k=384),
                  di["sd_m"][:].rearrange("n p k -> p n k"))

    # warm-up collective: absorbs the one-time CC rendezvous lead so the
    # real J AllReduce starts with a short lead
    wsb = ptile((1, 128), "wsb")
    V.memset(wsb[:], 0.0)
    warm_i = dram.tile([1, 128], F32, tag="warm_i")
    warm_o = dram.tile([1, 128], F32, tag="warm_o")
    DMA.dma_start(warm_i[:], wsb[:])
    G.collective_compute("AllReduce", ALU.add,
                         replica_groups=[list(range(NCORES))],
                         ins=[warm_i[:].opt()], outs=[warm_o[:].opt()])
    # early zero-fills (vector queue; keeps the gpsimd queue free for CC)
    jsb2 = ptile((21, 384), "jsb2")
    V.memset(jsb2[:], 0.0)
    rhs_f = persist.tile([11, 2304], BF16, tag="rhs_f", name="rhs_f")
    V.memset(rhs_f[:], 0.0)
    rot_all = ptile((B, NJ_ALL * 9), "rot_all")
    ra3 = rot_all[:].rearrange("p (j x) -> p j x", x=9)
    ra4 = rot_all[:].rearrange("p (j m n) -> p j m n", m=3, n=3)
    V.memset(rot_all[:], 0.0)
    V.memset(ra3[:, :, 0:9:4], 1.0)
    jall = ptile((96, 400), "jall")

    # ---------------- stage A: blend shapes + J partials ------------------
    jpt = jpool.tile([128, 512], F32, tag="jpsum", name="jpt")
    jps = jpt[:, 0:384]
    jpt2 = jpool.tile([128, 512], F32, tag="jpsum2", name="jpt2")
    jps_f = jpt2[:, 0:384]
    jps_m = jpt2[:, 384:385]

    vp_sbuf = [persist.tile([128, 384], BF16, tag=f"vp{i}", name=f"vp{i}")
               for i in range(NCH)]
    vpf_sbuf = [persist.tile([128, 384], BF16, tag=f"vpf{h}", name=f"vpf{h}")
                for h in range(3)]
    vpm_sbuf = [persist.tile([128, 384], BF16, tag=f"vpm{h}", name=f"vpm{h}")
                for h in range(2)]

    def sd_mms(pp, slab_t, betaT, last=True):
        for c3 in range(3):
            for lk in range(3):
                T.matmul(pp[:, c3 * 128:(c3 + 1) * 128],
                         slab_t[:, (c3 * 3 + lk) * 128:(c3 * 3 + lk + 1) * 128],
                         betaT[:, lk * 128:(lk + 1) * 128],
                         start=(lk == 0), stop=(lk == 2 and last))

    vsb = [persist.tile([128, 384], BF16, tag=f"vsb{i}", name=f"vsb{i}")
           for i in range(NCH)]
    vsf32 = {i: ptile((128, 384), f"vsf32{i}") for i in CH_PLAIN}

    # ---- A1: shape blend + J partials (everything the AllReduce needs) ----
    for i in range(NCH):
        sdt = slabs.tile((128, 1152), BF16, tag="sd_s")
        DMA.dma_start(sdt[:], di["sd_s"][i])
        pp = acc.tile([128, 384], F32, tag="vppsum", padded_shape=[128, 512])
        sd_mms(pp, sdt, betaT_s)
        S.copy(vsb[i][:], pp[:])
        if i in CH_PLAIN:
            V.tensor_copy(vsf32[i][:], pp[:])
        T.matmul(jps[0:55, :], jr_all[:, i * 55:(i + 1) * 55], vsb[i][:],
                 start=(i == 0), stop=(i == NCH - 1))

    # ---- AR1: smplx J AllReduce (launched before flame/mano A1) ----------
    jsb = ptile((55, 384), "jsb")
    S.copy(jsb[:], jps[0:55, :])
    ar_in1 = dram.tile([55, 384], F32, tag="ar_in1")
    ar_out1 = dram.tile([55, 384], F32, tag="ar_out1")
    DMA.dma_start(ar_in1[:], jsb[:])
    G.collective_compute("AllReduce", ALU.add,
                         replica_groups=[list(range(NCORES))],
                         ins=[ar_in1[:].opt()], outs=[ar_out1[:].opt()])
    arr = ptile((55, 384), "arr")
    G.dma_start(arr[:], ar_out1[:])

    for k in range(NFE_CH):
        sdt = slabs.tile((128, 1152), BF16, tag="sd_f")
        DMA.dma_start(sdt[:], di["sd_f"][k])
        pp = acc.tile([128, 384], F32, tag="vppsum", padded_shape=[128, 512])
        sd_mms(pp, sdt, betaT_f)
        vsf = slabs.tile((128, 384), BF16, tag="vsf")
        S.copy(vsf[:], pp[:])
        T.matmul(jps_f[0:5, 0:384], jrf_all[:, k * 5:(k + 1) * 5], vsf[:],
                 start=(k == 0), stop=(k == NFE_CH - 1))

    for k in range(NMJ_CH):
        pp = acc.tile([128, 384], F32, tag="vppsum", padded_shape=[128, 512])
        T.matmul(pp[:, 0:1], sdmj_all[:, k * 128:(k + 1) * 128], betam[:],
                 start=True, stop=True)
        vsm = slabs.tile((128, 1), BF16, tag="vsmj")
        S.copy(vsm[:], pp[:, 0:1])
        T.matmul(jps_m[0:48, 0:1], jrm_all[:, k * 48:(k + 1) * 48], vsm[:],
                 start=(k == 0), stop=(k == NMJ_CH - 1))

    # ---- AR2: flame + mano J AllReduce -----------------------------------
    S.copy(jsb2[0:5, :], jps_f[0:5, 0:384])
    jsb_m = ptile((48, 1), "jsb_m")
    S.copy(jsb_m[:], jps_m[0:48, 0:1])
    jpool_cm.__exit__(None, None, None)
    ar_in2 = dram.tile([21, 384], F32, tag="ar_in2")
    ar_out2 = dram.tile([21, 384], F32, tag="ar_out2")
    DMA.dma_start(ar_in2[:], jsb2[:])
    DMA.dma_start(ar_in2[5:21, 0:3], jsb_m[:])
    G.collective_compute("AllReduce", ALU.add,
                         replica_groups=[list(range(NCORES))],
                         ins=[ar_in2[:].opt()], outs=[ar_out2[:].opt()])
    G.dma_start(jall[55:60, 0:384], ar_out2[0:5, :])
    G.dma_start(jall[64:80, 384:387], ar_out2[5:21, 0:3])
    G.dma_start(jall[80:96, 384:387], ar_out2[5:21, 0:3])

    # ---- eyelid rows of rhs_f (early: only needs aux) --------------------
    epp = ptile((B, 2), "epp")
    V.tensor_mul(epp[:], aux[:, 3:5], aux[:, 0:1].broadcast_to([B, 2]))
    epT = persist.tile([2, 128], BF16, tag="epT", name="epT")
    ppe = acct.tile([128, 384], F32, tag="tpose", padded_shape=[128, 512])
    T.matmul(ppe[0:2, 0:128], epp[:, :], ident[:], is_transpose=True,
             start=True, stop=True)
    S.copy(epT[:], ppe[0:2, 0:128])
    for m3 in range(3):
        DMA.dma_start(rhs_f[5 + m3:6 + m3, (12 + m3) * 128:(13 + m3) * 128],
                      epT[1:2, :])
        DMA.dma_start(rhs_f[8 + m3:9 + m3, (15 + m3) * 128:(16 + m3) * 128],
                      epT[0:1, :])

    # ---------------- rodrigues (vector; overlaps A1 on tensor) -----------
    rot = ptile((B, NROT * 9), "rot")
    _rodrigues(nc, aa, rot, ptile)
    rot4 = rot[:].rearrange("p (j x) -> p j x", x=9)

    def pf_make(name, j0, n):
        t = ptile((B, n * 9), name)
        t9 = t[:].rearrange("p (j x) -> p j x", x=9)
        V.tensor_copy(t9, rot4[:, j0:j0 + n, :])
        V.tensor_scalar_add(t9[:, :, 0:9:4], t9[:, :, 0:9:4], -1.0)
        return t

    pf_s = pf_make("pf_s", 1, 21)
    pf_f = pf_make("pf_f", 22, 3)
    pf_m = [pf_make("pf_l", 25, 15), pf_make("pf_r", 40, 15)]

    def transpose_to(dst_ap, src_ap):
        pp = acct.tile([128, 384], F32, tag="tpose", padded_shape=[128, 512])
        k, n = src_ap.shape[0], src_ap.shape[1]
        T.matmul(pp[:n, :k], src_ap, ident[:k, :k], is_transpose=True,
                 start=True, stop=True)
        S.copy(dst_ap, pp[:n, :k])

    pfT_s_a = persist.tile([128, 128], BF16, tag="pfT_s_a", name="pfT_s_a")
    pfT_s_b = persist.tile([PD_S_K - 128, 128], BF16, tag="pfT_s_b", name="pfT_s_b")
    transpose_to(pfT_s_a[:], pf_s[:, 0:128])
    transpose_to(pfT_s_b[:], pf_s[:, 128:PD_S_K])
    pfT_f = persist.tile([PD_F_K, 128], BF16, tag="pfT_f", name="pfT_f")
    transpose_to(pfT_f[:], pf_f[:, :])
    pfT_m_a = [persist.tile([128, 128], BF16, tag="pfT_l_a", name="pfT_l_a"), persist.tile([128, 128], BF16, tag="pfT_r_a", name="pfT_r_a")]
    pfT_m_b = [persist.tile([PD_M_K - 128, 128], BF16, tag="pfT_l_b", name="pfT_l_b"),
               persist.tile([PD_M_K - 128, 128], BF16, tag="pfT_r_b", name="pfT_r_b")]
    for h in range(2):
        transpose_to(pfT_m_a[h][:], pf_m[h][:, 0:128])
        transpose_to(pfT_m_b[h][:], pf_m[h][:, 128:PD_M_K])

    # ---- world rotations (vector; independent of the AllReduce) ----------
    V.tensor_copy(ra3[:, 0:22, :], rot4[:, 0:22, :])
    V.tensor_copy(ra3[:, 57:60, :], rot4[:, 22:25, :])
    V.tensor_copy(ra3[:, 61:76, :], rot4[:, 25:40, :])
    V.tensor_copy(ra3[:, 77:92, :], rot4[:, 40:55, :])
    negid = persist.tile([55, 55], BF16, tag="negid", name="negid")
    V.tensor_scalar_mul(negid[:], ident[0:55, 0:55], -1.0)
    ones3 = persist.tile([3, 16], BF16, tag="ones3", name="ones3")
    V.memset(ones3[:], 1.0)

    Rw = ptile((B, NJ_ALL * 9), "Rw")
    Rw4 = Rw[:].rearrange("p (j m n) -> p j m n", m=3, n=3)
    fk_scr = ptile((B, 16 * 9), "fk_scr")

    def rw_mul(dst_sl, par_sl, loc_sl, n, par_bcast=False):
        dst = Rw4[:, dst_sl]
        par = Rw4[:, par_sl]
        if par_bcast:
            par = par.broadcast_to([B, n, 3, 3])
        loc = ra4[:, loc_sl]
        sc = fk_scr[:].rearrange("p (j m n) -> p j m n", m=3, n=3)[:, :n]
        for k in range(3):
            a_k = par[:, :, :, k:k + 1].broadcast_to([B, n, 3, 3])
            t_k = loc[:, :, k:k + 1, :].broadcast_to([B, n, 3, 3])
            if k == 0:
                V.tensor_mul(dst, a_k, t_k)
            else:
                V.tensor_mul(sc, a_k, t_k)
                V.tensor_add(dst, dst, sc)

    V.tensor_copy(Rw4[:, 0:1], ra4[:, 0:1])
    rw_mul(slice(1, 4), slice(0, 1), slice(1, 4), 3, par_bcast=True)
    rw_mul(slice(4, 7), slice(1, 4), slice(4, 7), 3)
    rw_mul(slice(7, 10), slice(4, 7), slice(7, 10), 3)
    rw_mul(slice(10, 13), slice(7, 10), slice(10, 13), 3)
    rw_mul(slice(13, 15), slice(9, 10), slice(13, 15), 2, par_bcast=True)
    rw_mul(slice(15, 18), slice(12, 15), slice(15, 18), 3)
    rw_mul(slice(18, 20), slice(16, 18), slice(18, 20), 2)
    V.tensor_copy(Rw4[:, 22:25], Rw4[:, 15:16].broadcast_to([B, 3, 3, 3]))
    rw_mul(slice(20, 22), slice(18, 20), slice(20, 22), 2)
    V.tensor_copy(Rw4[:, 25:40], Rw4[:, 20:21].broadcast_to([B, 15, 3, 3]))
    V.tensor_copy(Rw4[:, 40:55], Rw4[:, 21:22].broadcast_to([B, 15, 3, 3]))
    # flame roots/jaw/eyes + mano roots and level-1 (parents are identity)
    V.tensor_copy(Rw4[:, 55:61], ra4[:, 55:61])
    V.tensor_copy(Rw4[:, 76:77], ra4[:, 76:77])
    V.tensor_copy(Rw4[:, 61:74:3], ra4[:, 61:74:3])
    V.tensor_copy(Rw4[:, 77:90:3], ra4[:, 77:90:3])
    rw_mul(slice(62, 75, 3), slice(61, 74, 3), slice(62, 75, 3), 5)
    rw_mul(slice(78, 91, 3), slice(77, 90, 3), slice(78, 91, 3), 5)
    rw_mul(slice(63, 76, 3), slice(62, 75, 3), slice(63, 76, 3), 5)
    rw_mul(slice(79, 92, 3), slice(78, 91, 3), slice(79, 92, 3), 5)

    # ---- A2: posedirs + flame/mano v_posed (overlaps the AllReduce) ------
    for i in range(NCH):
        pda = slabs.tile((128, 384), BF16, tag="pd_s_a")
        pdb = slabs.tile((PD_S_K - 128, 384), BF16, tag="pd_s_b")
        DMA.dma_start(pda[:], di["pd_s_a"][i])
        DMA.dma_start(pdb[:], di["pd_s_b"][i])
        pq = acc.tile([128, 384], F32, tag="vppsum", padded_shape=[128, 512])
        for c3 in range(3):
            T.matmul(pq[:, c3 * 128:(c3 + 1) * 128],
                     pda[:, c3 * 128:(c3 + 1) * 128], pfT_s_a[:],
                     start=True, stop=False)
            T.matmul(pq[:, c3 * 128:(c3 + 1) * 128],
                     pdb[:, c3 * 128:(c3 + 1) * 128], pfT_s_b[:],
                     start=False, stop=True)
        if i in CH_PLAIN:
            V.tensor_add(vp_sbuf[i][:], vsf32[i][:], pq[:])
        else:
            S.copy(vp_sbuf[i][:], pq[:])

    for h in range(3):
        sdt = slabs.tile((128, 1152), BF16, tag="sd_f")
        DMA.dma_start(sdt[:], di["sd_f"][NFE_CH + h])
        pp = acc.tile([128, 384], F32, tag="vppsum", padded_shape=[128, 512])
        pdf = slabs.tile((PD_F_K, 384), BF16, tag="pd_f")
        DMA.dma_start(pdf[:], di["pd_f"][h])
        for c3 in range(3):
            for lk in range(3):
                T.matmul(pp[:, c3 * 128:(c3 + 1) * 128],
                         sdt[:, (c3 * 3 + lk) * 128:(c3 * 3 + lk + 1) * 128],
                         betaT_f[:, lk * 128:(lk + 1) * 128],
                         start=(lk == 0), stop=False)
            T.matmul(pp[:, c3 * 128:(c3 + 1) * 128],
                     pdf[:, c3 * 128:(c3 + 1) * 128], pfT_f[:],
                     start=False, stop=True)
        S.copy(vpf_sbuf[h][:], pp[:])

    for h in range(2):
        pps = acc.tile([128, 384], F32, tag="vppsum", padded_shape=[128, 512])
        for c3 in range(3):
            T.matmul(pps[:, c3:c3 + 1], sdm_all[:, h * 384 + c3 * 128:h * 384 + (c3 + 1) * 128],
                     betam[:], start=True, stop=True)
        vshm = ptile((128, 3), f"vshm{h}")
        S.copy(vshm[:], pps[:, 0:3])
        pda = slabs.tile((128, 384), BF16, tag="pd_m_a")
        pdb = slabs.tile((PD_M_K - 128, 384), BF16, tag="pd_m_b")
        DMA.dma_start(pda[:], di["pd_m_a"][h])
        DMA.dma_start(pdb[:], di["pd_m_b"][h])
        pq = acc.tile([128, 384], F32, tag="vppsum", padded_shape=[128, 512])
        for c3 in range(3):
            T.matmul(pq[:, c3 * 128:(c3 + 1) * 128],
                     pda[:, c3 * 128:(c3 + 1) * 128], pfT_m_a[h][:],
                     start=True, stop=False)
            T.matmul(pq[:, c3 * 128:(c3 + 1) * 128],
                     pdb[:, c3 * 128:(c3 + 1) * 128], pfT_m_b[h][:],
                     start=False, stop=True)
        vpm = vpm_sbuf[h]
        V.tensor_add(vpm[:].rearrange("p (c b) -> p c b", b=128),
                     pq[:].rearrange("p (c b) -> p c b", b=128),
                     vshm[:].unsqueeze(2).broadcast_to([128, 3, 128]))

    # ================= joints + A_rel assembly (post-AllReduce) ===========
    arr3 = arr  # smplx J sum from AR1
    V.tensor_add(jall[0:55, 0:384], arr3[:], joffT[:])
    # broadcast compact mano J into (c,b) layout on an aligned scratch tile,
    # then DMA into jall rows 60:92 (engine partition starts must be 32-aligned)
    jmtmp = ptile((32, 384), "jmtmp")
    V.tensor_copy(jmtmp[:].rearrange("p (c b) -> p c b", b=128),
                  jall[64:96, 384:387].unsqueeze(2).broadcast_to([32, 3, 128]))
    G.dma_start(jall[60:92, 0:384], jmtmp[:])

    # rel = mrel_all @ J (one fp32 matmul over the whole forest)
    ppr = acct.tile([128, 384], F32, tag="tpose", padded_shape=[128, 512])
    T.matmul(ppr[0:92, 0:384], mrelT_all[:], jall[0:92, 0:384],
             start=True, stop=True)
    rel_all = ptile((92, 384), "rel_all")
    S.copy(rel_all[:], ppr[0:92, 0:384])

    # batch-major J and rel:  jrb[:, 0:276] = J (c-major), [:, 280:556] = rel
    jrb = ptile((B, 560), "jrb")
    for c3 in range(3):
        ppj = acct.tile([128, 384], F32, tag="tpose", padded_shape=[128, 512])
        T.matmul(ppj[0:128, 0:92], jall[0:92, c3 * 128:(c3 + 1) * 128],
                 ident[0:92, 0:92], is_transpose=True, start=True, stop=True)
        T.matmul(ppj[0:128, 192:284], rel_all[:, c3 * 128:(c3 + 1) * 128],
                 ident[0:92, 0:92], is_transpose=True, start=True, stop=True)
        S.copy(jrb[:].rearrange("p (t x) -> p t x", x=280)[:, :, c3 * 92:(c3 + 1) * 92],
               ppj[:].rearrange("p (t x) -> p t x", x=192)[:, :, 0:92])

    jbv = jrb[:, 0:276].rearrange("p (c a) -> p c a", c=3)
    relv = jrb[:, 280:556].rearrange("p (c a) -> p c a", c=3)

    # q_a = R_a^T rel_a ; c_a = R^w_a q_a ; u_a = R^w_a J_a   (all joints)
    qv = ptile((B, 3 * NJ_ALL), "qv")
    cv = ptile((B, 3 * NJ_ALL), "cv")
    uv = ptile((B, 3 * NJ_ALL), "uv")
    scr3 = ptile((B, 3 * NJ_ALL), "scr3")
    q3 = qv[:].rearrange("p (a k) -> p a k", k=3)
    c3v = cv[:].rearrange("p (a k) -> p a k", k=3)
    u3 = uv[:].rearrange("p (a k) -> p a k", k=3)
    s3 = scr3[:].rearrange("p (a k) -> p a k", k=3)
    for m in range(3):
        rm = relv[:, m, :].unsqueeze(2).broadcast_to([B, NJ_ALL, 3])
        if m == 0:
            V.tensor_mul(q3, ra4[:, :, 0, :], rm)
        else:
            V.tensor_mul(s3, ra4[:, :, m, :], rm)
            V.tensor_add(q3, q3, s3)
    for k in range(3):
        qk = q3[:, :, k].unsqueeze(2).broadcast_to([B, NJ_ALL, 3])
        if k == 0:
            V.tensor_mul(c3v, Rw4[:, :, :, 0], qk)
        else:
            V.tensor_mul(s3, Rw4[:, :, :, k], qk)
            V.tensor_add(c3v, c3v, s3)
    gscr = ptile((B, 3 * NJ_ALL), "gscr")
    g3 = gscr[:].rearrange("p (a k) -> p a k", k=3)
    for k in range(3):
        jk = jbv[:, k, :].unsqueeze(2).broadcast_to([B, NJ_ALL, 3])
        if k == 0:
            G.tensor_mul(u3, Rw4[:, :, :, 0], jk)
        else:
            G.tensor_mul(g3, Rw4[:, :, :, k], jk)
            G.tensor_add(u3, u3, g3)

    # ---- scale / mirror folding (batch-major) ----------------------------
    V.tensor_scalar_mul(Rw[:, 495:540], Rw[:, 495:540], aux[:, 0:1])
    V.tensor_scalar_mul(cv[:, 165:180], cv[:, 165:180], aux[:, 0:1])
    V.tensor_scalar_mul(uv[:, 165:180], uv[:, 165:180], aux[:, 0:1])
    negls = ptile((B, 1), "negls")
    V.tensor_scalar_mul(negls[:], aux[:, 1:2], -1.0)
    V.tensor_scalar_mul(Rw4[:, 60:76, 0, :], Rw4[:, 60:76, 0, :], negls[:, 0:1])
    V.tensor_scalar_mul(Rw4[:, 60:76, 1:3, :], Rw4[:, 60:76, 1:3, :], aux[:, 1:2])
    V.tensor_scalar_mul(c3v[:, 60:76, 0], c3v[:, 60:76, 0], negls[:, 0:1])
    V.tensor_scalar_mul(c3v[:, 60:76, 1:3], c3v[:, 60:76, 1:3], aux[:, 1:2])
    V.tensor_scalar_mul(u3[:, 60:76, 0], u3[:, 60:76, 0], negls[:, 0:1])
    V.tensor_scalar_mul(u3[:, 60:76, 1:3], u3[:, 60:76, 1:3], aux[:, 1:2])
    V.tensor_scalar_mul(Rw[:, 684:828], Rw[:, 684:828], aux[:, 2:3])
    V.tensor_scalar_mul(cv[:, 228:276], cv[:, 228:276], aux[:, 2:3])
    V.tensor_scalar_mul(uv[:, 228:276], uv[:, 228:276], aux[:, 2:3])

    # ---- per-batch bias vectors (head / left / right) --------------------
    bias9 = ptile((B, 9), "bias9")
    hm = ptile((B, 6), "hm")
    hl = ptile((B, 3), "hl")
    hr = ptile((B, 3), "hr")
    V.tensor_add(hm[:, 0:3], jbv[:, :, 23], jbv[:, :, 24])
    V.tensor_add(hm[:, 3:6], jbv[:, :, 58], jbv[:, :, 59])
    V.tensor_sub(bias9[:, 0:3], hm[:, 0:3], hm[:, 3:6])
    V.tensor_scalar_mul(bias9[:, 0:3], bias9[:, 0:3], 0.5)
    V.tensor_add(bias9[:, 0:3], bias9[:, 0:3], aux[:, 5:8])
    V.tensor_sub(hl[:], aux[:, 8:11], jbv[:, :, 60])
    V.tensor_sub(bias9[:, 3:4], jbv[:, 0:1, 20], hl[:, 0:1])
    V.tensor_add(bias9[:, 4:6], jbv[:, 1:3, 20], hl[:, 1:3])
    V.tensor_sub(hr[:], aux[:, 11:14], jbv[:, :, 60])
    V.tensor_add(bias9[:, 6:9], jbv[:, :, 21], hr[:])

    # bsT per group -> [1, 384] bias rhs rows (DMA reshapes [3,128]->[1,384])
    rbias = []
    for g in range(3):
        ppb = acct.tile([128, 384], F32, tag="tpose", padded_shape=[128, 512])
        T.matmul(ppb[0:3, 0:128], bias9[:, g * 3:(g + 1) * 3], ident[:],
                 is_transpose=True, start=True, stop=True)
        bst = persist.tile([3, 128], BF16, tag=f"bsT{g}", name=f"bsT{g}")
        S.copy(bst[:], ppb[0:3, 0:128])
        rb = persist.tile([1, 384], BF16, tag=f"rbias{g}", name=f"rbias{g}")
        G.dma_start(rb[:], bst[:])
        rbias.append(rb)

    # ---- rhs assembly ----------------------------------------------------
    rhs_s = persist.tile([55, 1536], BF16, tag="rhs_s", name="rhs_s")
    rhs_m = [persist.tile([16, 1536], BF16, tag="rhs_l", name="rhs_l"),
             persist.tile([16, 1536], BF16, tag="rhs_r", name="rhs_r")]
    groups = [(0, 55, rhs_s, di["ancT_s"], None),
              (55, 5, rhs_f, di["ancT_f"], 0),
              (60, 16, rhs_m[0], di["ancT_m"], 1),
              (76, 16, rhs_m[1], di["ancT_m"], 2)]
    ancT_t = {}
    for nm in ("ancT_s", "ancT_f", "ancT_m"):
        n = di[nm].shape[0]
        t = persist.tile([n, n], BF16, tag=nm, name=nm + "_t")
        DMA.dma_start(t[:], di[nm][:])
        ancT_t[nm] = t

    for (a0, ng, rhs_t, anc_d, bias_g) in groups:
        anc_t = ancT_t["ancT_s" if ng == 55 else ("ancT_f" if ng == 5 else "ancT_m")]
        # cT/uT via strided transposes:  cuT[:, 0:384]=c^T, [:, 384:768]=u^T
        cuT = persist.tile([ng, 768], BF16, tag=f"cuT{a0}", name=f"cuT{a0}")
        for m in range(3):
            ppc = acct.tile([128, 384], F32, tag="tpose", padded_shape=[128, 512])
            T.matmul(ppc[0:ng, 0:128],
                     cv[:, 3 * a0 + m:3 * (a0 + ng - 1) + m + 1:3],
                     ident[:], is_transpose=True, start=True, stop=True)
            T.matmul(ppc[0:ng, 192:320],
                     uv[:, 3 * a0 + m:3 * (a0 + ng - 1) + m + 1:3],
                     ident[:], is_transpose=True, start=True, stop=True)
            S.copy(cuT[0:ng].rearrange("p (t x) -> p t x", x=384)[:, :, m * 128:(m + 1) * 128],
                   ppc[0:ng].rearrange("p (t x) -> p t x", x=192)[:, :, 0:128])
        # translations: Anc @ c - u (+ bias) -> rhs cols 1152:1536
        ppt = acct.tile([128, 384], F32, tag="tpose", padded_shape=[128, 512])
        T.matmul(ppt[0:ng, 0:384], anc_t[:], cuT[0:ng, 0:384],
                 start=True, stop=False)
        T.matmul(ppt[0:ng, 0:384], negid[0:ng, 0:ng], cuT[0:ng, 384:768],
                 start=False, stop=(bias_g is None))
        if bias_g is not None:
            T.matmul(ppt[0:ng, 0:384], ones3[0:1, 0:ng], rbias[bias_g][:],
                     start=False, stop=True)
        S.copy(rhs_t[0:ng, 1152:1536], ppt[0:ng, 0:384])
        # rotations: R^w columns -> rhs cols n*384 + m*128
        for n4 in range(3):
            ppn = acct.tile([128, 384], F32, tag="tpose", padded_shape=[128, 512])
            for m in range(3):
                T.matmul(ppn[0:ng, m * 128:(m + 1) * 128],
                         Rw[:, 9 * a0 + m * 3 + n4:9 * (a0 + ng - 1) + m * 3 + n4 + 1:9],
                         ident[:], is_transpose=True, start=True, stop=True)
            S.copy(rhs_t[0:ng, n4 * 384:(n4 + 1) * 384], ppn[0:ng, 0:384])

    acct_cm.__exit__(None, None, None)
    acc_cm.__exit__(None, None, None)
    big_cm = tc.tile_pool(name="big", bufs=2, space="PSUM")
    big = big_cm.__enter__()

    # ---------------- skinning per chunk (bf16 DVE fast-path) -------------
    def t_apply(dst_ap, tp_bf, x_sbuf, scratch):
        """dst = sum_{n<3} T'[n]*x_n + T'[3]; bf16 SBUF operands."""
        d3 = dst_ap.rearrange("p (m b) -> p m b", b=128)
        x3 = x_sbuf[:].rearrange("p (c b) -> p c b", b=128)
        tp = tp_bf[:].rearrange("p (n m b) -> p n m b", m=3, b=128)
        sc = scratch[:].rearrange("p (n m b) -> p n m b", m=3, b=128)
        V.tensor_mul(sc[:, 0:3], tp[:, 0:3],
                     x3[:].unsqueeze(2).broadcast_to([128, 3, 3, 128]))
        V.tensor_add(sc[:, 0], sc[:, 0], sc[:, 1])
        V.tensor_add(sc[:, 0], sc[:, 0], tp[:, 3])
        V.tensor_add(d3, sc[:, 0], sc[:, 2])

    scr_t = [persist.tile([128, 1152], BF16, tag=f"scr{i}", name=f"scr{i}")
             for i in range(4)]

    for i in range(NCH):
        if CH_HEAD0 <= i < CH_HEAD0 + 3:
            h = i - CH_HEAD0
            hv = slabs.tile((128, 384), BF16, tag="hv", bufs=2, name="hv")
            wt = wre_all[:, h * 128:(h + 1) * 128]
            tp1 = big.tile([128, 1536], F32, tag="bigp")
            for g, w in ((0, 512), (1, 512), (2, 128)):
                T.matmul(tp1[:, g * 512:g * 512 + w], wt,
                         rhs_f[:, g * 512:g * 512 + w], start=True, stop=True)
            tp2 = big.tile([128, 1536], F32, tag="bigp")
            for g, w in ((0, 512), (1, 512), (2, 128)):
                T.matmul(tp2[:, g * 512:g * 512 + w], wt,
                         rhs_f[:, 1152 + g * 512:1152 + g * 512 + w],
                         start=True, stop=True)
            tb1 = slabs.tile((128, 1152), BF16, tag="tpb1", bufs=2, name="tb1")
            S.copy(tb1[:], tp1[:, 0:1152])
            tb2 = slabs.tile((128, 1152), BF16, tag="tpb2", bufs=2, name="tb2")
            V.tensor_copy(tb2[:], tp2[:, 0:1152])
            d3 = hv[:].rearrange("p (m b) -> p m b", b=128)
            x3 = vpf_sbuf[h][:].rearrange("p (c b) -> p c b", b=128)
            t1 = tb1[:].rearrange("p (n m b) -> p n m b", m=3, b=128)
            t2 = tb2[:].rearrange("p (n m b) -> p n m b", m=3, b=128)
            sc = scr_t[i % 4][:].rearrange("p (n m b) -> p n m b", m=3, b=128)
            V.tensor_mul(sc[:, 0:3], t1[:, 0:3],
                         x3[:].unsqueeze(2).broadcast_to([128, 3, 3, 128]))
            V.tensor_add(sc[:, 0], sc[:, 0], sc[:, 1])
            V.tensor_add(sc[:, 0], sc[:, 0], sc[:, 2])
            V.tensor_add(sc[:, 1], t2[:, 0], t2[:, 1])
            V.tensor_add(sc[:, 1], sc[:, 1], t2[:, 2])
            V.tensor_add(d3, sc[:, 0], sc[:, 1])
            V.tensor_add(vp_sbuf[i][:], vp_sbuf[i][:], hv[:])
        elif i in (CH_HL, CH_HR):
            h = i - CH_HL
            hv = slabs.tile((128, 384), BF16, tag="hv", bufs=2, name="hv")
            wt = wm_all[:, h * 128:(h + 1) * 128]
            tpm = big.tile([128, 1536], F32, tag="bigp")
            for g in range(3):
                T.matmul(tpm[:, g * 512:(g + 1) * 512], wt,
                         rhs_m[h][:, g * 512:(g + 1) * 512], start=True, stop=True)
            tbm = slabs.tile((128, 1536), BF16, tag="tpbm", bufs=2, name="tbm")
            S.copy(tbm[:], tpm[:])
            t_apply(hv[:], tbm, vpm_sbuf[h], scr_t[i % 4])
            V.tensor_add(vp_sbuf[i][:], vp_sbuf[i][:], hv[:])

        wt = w_all[:, i * 128:(i + 1) * 128]
        tps = big.tile([128, 1536], F32, tag="bigp")
        for g in range(3):
            T.matmul(tps[:, g * 512:(g + 1) * 512], wt,
                     rhs_s[:, g * 512:(g + 1) * 512], start=True, stop=True)
        tbs = slabs.tile((128, 1536), BF16, tag="tpbs", bufs=3, name="tbs")
        S.copy(tbs[:], tps[:])
        ot = slabs.tile((128, 384), F32, tag="outt", bufs=3, name="ot")
        t_apply(ot[:], tbs, vp_sbuf[i], scr_t[i % 4])
        DMA.dma_start(out_d[i * 128:(i + 1) * 128, :], ot[:])

    big_cm.__exit__(None, None, None)
    es.close()


def _rodrigues(nc, aa, rot, ptile):
    V, S = nc.vector, nc.scalar
    J = NROT
    aa3 = aa[:].rearrange("p (j k) -> p j k", k=3)
    sq = ptile((B, J), "rg_sq")
    tmp = ptile((B, J), "rg_tmp")
    V.tensor_mul(sq[:], aa3[:, :, 0], aa3[:, :, 0])
    V.tensor_mul(tmp[:], aa3[:, :, 1], aa3[:, :, 1])
    V.tensor_add(sq[:], sq[:], tmp[:])
    V.tensor_mul(tmp[:], aa3[:, :, 2], aa3[:, :, 2])
    V.tensor_add(sq[:], sq[:], tmp[:])
    eps_t = ptile((B, 1), "rg_eps")
    nc.gpsimd.memset(eps_t[:], 1e-8)
    hpi_t = ptile((B, 1), "rg_hpi")
    nc.gpsimd.memset(hpi_t[:], float(np.pi / 2))
    zero_t = ptile((B, 1), "rg_zero")
    nc.gpsimd.memset(zero_t[:], 0.0)
    ang = ptile((B, J), "rg_ang")
    S.activation(ang[:], sq[:], AF.Sqrt, bias=eps_t[:])
    inv = ptile((B, J), "rg_inv")
    V.reciprocal(inv[:], ang[:])
    sn = ptile((B, J), "rg_sin")
    co = ptile((B, J), "rg_cos")
    S.activation(sn[:], ang[:], AF.Sin, bias=zero_t[:])
    S.activation(co[:], ang[:], AF.Sin, bias=hpi_t[:])
    nv = ptile((B, 3 * J), "rg_n")
    n3 = nv[:].rearrange("p (j k) -> p j k", k=3)
    V.tensor_mul(n3, aa3, inv[:].unsqueeze(2).broadcast_to([B, J, 3]))
    u = ptile((B, J), "rg_u")
    V.tensor_scalar(u[:], co[:], -1.0, 1.0, ALU.mult, ALU.add)
    un = ptile((B, 3 * J), "rg_un")
    un3 = un[:].rearrange("p (j k) -> p j k", k=3)
    V.tensor_mul(un3, n3, u[:].unsqueeze(2).broadcast_to([B, J, 3]))
    q = ptile((B, 3 * J), "rg_q")
    q3 = q[:].rearrange("p (j k) -> p j k", k=3)
    V.tensor_mul(q3, un3, n3)
    d = ptile((B, J), "rg_d")
    V.tensor_add(d[:], q3[:, :, 0], q3[:, :, 1])
    V.tensor_add(d[:], d[:], q3[:, :, 2])
    dd = ptile((B, J), "rg_dd")
    V.tensor_scalar(dd[:], d[:], -1.0, 1.0, ALU.mult, ALU.add)
    snv = ptile((B, 3 * J), "rg_snv")
    s3 = snv[:].rearrange("p (j k) -> p j k", k=3)
    V.tensor_mul(s3, n3, sn[:].unsqueeze(2).broadcast_to([B, J, 3]))
    r4 = rot[:].rearrange("p (j m n) -> p j m n", m=3, n=3)
    for m in range(3):
        V.tensor_add(r4[:, :, m, m], q3[:, :, m], dd[:])
    p = ptile((B, J), "rg_p")
    V.tensor_mul(p[:], un3[:, :, 0], n3[:, :, 1])
    V.tensor_sub(r4[:, :, 0, 1], p[:], s3[:, :, 2])
    V.tensor_add(r4[:, :, 1, 0], p[:], s3[:, :, 2])
    V.tensor_mul(p[:], un3[:, :, 0], n3[:, :, 2])
    V.tensor_add(r4[:, :, 0, 2], p[:], s3[:, :, 1])
    V.tensor_sub(r4[:, :, 2, 0], p[:], s3[:, :, 1])
    V.tensor_mul(p[:], un3[:, :, 1], n3[:, :, 2])
    V.tensor_sub(r4[:, :, 1, 2], p[:], s3[:, :, 0])
    V.tensor_add(r4[:, :, 2, 1], p[:], s3[:, :, 0])


# ================================================================ entry

_CACHED = {}
DEBUG = False


def _get_nc():
    if "nc" not in _CACHED:
        _CACHED["nc"] = _build_nc()
    return _CACHED["nc"]


PROFILE = False
TRACE_DIR = None


def kernel(**inputs):
    in_maps, vid_all = _host_prep(inputs)
    nc = _get_nc()
    kw = {}
    if PROFILE and TRACE_DIR:
        kw["tmpdir"] = TRACE_DIR
    res = run_bass_kernel_spmd(nc, in_maps, core_ids=list(range(NCORES)),
                               trace=PROFILE, **kw)
    _CACHED["last_res"] = res
    out = np.zeros((B, VS, 3), np.float32)
    for c in range(NCORES):
        o = np.asarray(res.results[c]["out"]).reshape(ROWS, 3, B)
        vok = vid_all[c] >= 0
        out[:, vid_all[c][vok], :] = o[vok].transpose(2, 0, 1)
    return out



# revision 29
# speedup vs baseline: 1.0303x; 1.0303x over previous
"""EHM (SMPLX body + FLAME head + MANO hands) Bass kernel for 8 TRN2 NeuronCores.

Sharding: VERTEX sharding — model weights (shapedirs/posedirs/regressors/lbs
weights, ~130MB) dominate HBM traffic, so each core owns 1/8 of the SMPLX
vertices (plus the FLAME/MANO vertices its SMPLX rows stitch in) and computes
ALL B=128 batch elements for its shard.  The only cross-core dependency is the
joint regression J = J_regressor @ v_shaped -> one [76, 384] AllReduce of
partial joint sums.  FK (92 joints) is replicated on every core on the vector
engine with batch on partitions (B == 128 == n_partitions).

Per-vertex data layout: [vertex(partition<=128), (c, b)] with c-major free dim
(col = c*128 + b).  Batch-staged data (poses, FK, A matrices): [b(part), free].
"""

import sys

sys.path.insert(0, "/opt/trn_rl_repo")

from contextlib import ExitStack

import numpy as np
import ml_dtypes

BF16NP = ml_dtypes.bfloat16

import concourse.bass as bass
import concourse.bacc as bacc
import concourse.tile as tile
import concourse.mybir as mybir
from concourse.bass_utils import run_bass_kernel_spmd

F32 = mybir.dt.float32
BF16 = mybir.dt.bfloat16
AF = mybir.ActivationFunctionType
ALU = mybir.AluOpType

# ---------------------------------------------------------------- constants
B = 128
VS, VF, VM = 10475, 5023, 778
NL = 350
NCORES = 8

SMPLX_PARENTS = np.array([-1,0,0,0,1,2,3,4,5,6,7,8,9,9,9,12,13,14,16,17,18,19,
                          15,15,15,20,25,26,20,28,29,20,31,32,20,34,35,20,37,38,
                          21,40,41,21,43,44,21,46,47,21,49,50,21,52,53])
FLAME_PARENTS = np.array([-1,0,1,1,1])
MANO_PARENTS = np.array([-1,0,1,2,0,4,5,0,7,8,0,10,11,0,13,14])

N_PLAIN, N_HEAD, N_HL, N_HR = 768, 384, 128, 128
ROWS = N_PLAIN + N_HEAD + N_HL + N_HR        # 1408
NCH = ROWS // 128                            # 11
CH_PLAIN = set(range(0, 6))
CH_HEAD0 = 6                                 # chunks 6,7,8 head; 9 L; 10 R
CH_HL, CH_HR = 9, 10

NFE_CH = 5
NMJ_CH = 3
PD_S_K = 189
PD_F_K = 27
PD_M_K = 135

NJ_ALL = 92
OFF_S, OFF_F, OFF_L, OFF_R = 0, 55, 60, 76
NROT = 55
ROT_S0, ROT_F0, ROT_L0, ROT_R0 = 0, 22, 25, 40

BF16_INPUTS = {"w_s", "wre_f", "w_m", "ancT_s", "ancT_f", "ancT_m",
               "sd_s", "pd_s_a", "pd_s_b", "jr_s", "sd_f", "jr_f", "pd_f",
               "sd_m", "pd_m_a", "pd_m_b", "sd_mj", "jreg_m",
               "betaT_s", "betaT_f", "betam"}


def _fk_forest():
    par = np.empty(NJ_ALL, np.int64)
    par[OFF_S:OFF_S + 55] = SMPLX_PARENTS
    par[OFF_F:OFF_F + 5] = np.where(FLAME_PARENTS < 0, -1, FLAME_PARENTS + OFF_F)
    par[OFF_L:OFF_L + 16] = np.where(MANO_PARENTS < 0, -1, MANO_PARENTS + OFF_L)
    par[OFF_R:OFF_R + 16] = np.where(MANO_PARENTS < 0, -1, MANO_PARENTS + OFF_R)
    return par


def _fk_levels(par):
    depth = np.zeros(NJ_ALL, np.int64)
    for j in range(NJ_ALL):
        if par[j] >= 0:
            depth[j] = depth[par[j]] + 1
    levels = []
    for d in range(1, int(depth.max()) + 1):
        js = np.nonzero(depth == d)[0]
        runs, i = [], 0
        while i < len(js):
            j0, p0 = int(js[i]), int(par[js[i]])
            if i + 1 < len(js):
                ds = int(js[i + 1]) - j0
                ps = int(par[js[i + 1]]) - p0
            else:
                ds, ps = 1, 0
            n = 1
            while (i + n < len(js) and int(js[i + n]) == j0 + n * ds
                   and int(par[js[i + n]]) == p0 + n * ps):
                n += 1
            if n == 1:
                ds, ps = 1, 0
            runs.append((j0, ds, n, p0, ps))
            i += n
        levels.append(runs)
    return levels


# ================================================================ host prep

def _split_sizes(total, parts):
    q, r = divmod(total, parts)
    return [q + (1 if i < r else 0) for i in range(parts)]


def _pad_ids(ids, n):
    out = np.full(n, -1, np.int64)
    out[:len(ids)] = ids
    return out


def _host_prep(inp):
    f32 = np.float32
    s2f = np.asarray(inp["smplx2flame_ind"])
    head_ix = np.asarray(inp["head_index"])
    s2l = np.asarray(inp["smplx2mano_left"])
    s2r = np.asarray(inp["smplx2mano_right"])

    head_sv = s2f[head_ix]
    special = np.zeros(VS, bool)
    special[head_sv] = True
    special[s2l] = True
    special[s2r] = True
    plain_sv = np.nonzero(~special)[0]

    pl_sp = np.cumsum([0] + _split_sizes(len(plain_sv), NCORES))
    hd_sp = np.cumsum([0] + _split_sizes(len(head_ix), NCORES))
    hl_sp = np.cumsum([0] + _split_sizes(VM, NCORES))
    fe_sp = np.cumsum([0] + _split_sizes(VF, NCORES))
    mj_sp = np.cumsum([0] + _split_sizes(VM * 3, NCORES))

    sd_s_np = np.asarray(inp["smplx_shapedirs"], f32)
    pd_s_np = np.asarray(inp["smplx_posedirs"], f32)
    jr_s_np = np.asarray(inp["smplx_J_regressor"], f32)
    w_s_np = np.asarray(inp["smplx_lbs_weights"], f32)
    tmpl_s = np.asarray(inp["smplx_v_template"], f32)
    sd_f_np = np.asarray(inp["flame_shapedirs"], f32)
    pd_f_np = np.asarray(inp["flame_posedirs"], f32)
    jr_f_np = np.asarray(inp["flame_J_regressor"], f32)
    w_f_np = np.asarray(inp["flame_lbs_weights"], f32)
    tmpl_f = np.asarray(inp["flame_v_template"], f32)
    re_np = np.asarray(inp["r_eyelid"], f32)
    le_np = np.asarray(inp["l_eyelid"], f32)
    sd_m_np = np.asarray(inp["mano_shapedirs"], f32)
    pd_m_np = np.asarray(inp["mano_posedirs"], f32)
    jr_m_np = np.asarray(inp["mano_J_regressor"], f32)
    w_m_np = np.asarray(inp["mano_lbs_weights"], f32)
    tmpl_m = np.asarray(inp["mano_v_template"], f32)

    aa = np.concatenate([
        np.asarray(inp["global_pose"], f32).reshape(B, 3),
        np.asarray(inp["body_pose"], f32).reshape(B, 63),
        np.asarray(inp["jaw_params"], f32).reshape(B, 3),
        np.asarray(inp["eye_pose"], f32).reshape(B, 6),
        np.asarray(inp["left_hand_pose"], f32).reshape(B, 45),
        np.asarray(inp["right_hand_pose"], f32).reshape(B, 45),
    ], axis=1)

    ep = np.asarray(inp["eyelid_params"], f32)
    aux = np.concatenate([
        np.asarray(inp["head_scale"], f32)[:, None],
        np.asarray(inp["left_hand_scale"], f32)[:, None],
        np.asarray(inp["right_hand_scale"], f32)[:, None],
        ep[:, 0:1], ep[:, 1:2],
        np.asarray(inp["head_pos_offset"], f32),
        np.asarray(inp["left_hand_pos_offset"], f32),
        np.asarray(inp["right_hand_pos_offset"], f32),
    ], axis=1)                                               # [128, 14]

    def beta_T(second):
        b = np.concatenate([np.asarray(inp["shape_params"], f32), second], 1)
        bt = np.zeros((384, B), f32)
        bt[:NL] = b.T
        bt[NL] = 1.0
        return bt.reshape(3, 128, B)

    betaT_s = beta_T(np.asarray(inp["body_exp"], f32))
    betaT_f = beta_T(np.asarray(inp["flame_exp"], f32))

    joff = np.asarray(inp["joints_offset"], f32)
    joffT = np.ascontiguousarray(joff.transpose(1, 2, 0)).reshape(55, 384)

    def mrel_T(par, nj):
        m = np.eye(nj, dtype=f32)
        for j in range(1, nj):
            if par[j] >= 0:
                m[j, par[j]] = -1.0
        return np.ascontiguousarray(m.T)

    betam = np.zeros((11, 1), f32)
    betam[:10, 0] = np.asarray(inp["mano_betas"], f32)[0]
    betam[10, 0] = 1.0

    def anc_T(par, nj):
        m = np.zeros((nj, nj), f32)
        for j in range(nj):
            a = j
            while a >= 0:
                m[j, a] = 1.0
                a = par[a]
        return np.ascontiguousarray(m.T)

    fpar = _fk_forest()
    rep = dict(aa=aa, aux=aux, betaT_s=betaT_s, betaT_f=betaT_f, joffT=joffT,
               mrelT_all=mrel_T(fpar, NJ_ALL), betam=betam,
               ancT_s=anc_T(SMPLX_PARENTS, 55), ancT_f=anc_T(FLAME_PARENTS, 5),
               ancT_m=anc_T(MANO_PARENTS, 16),
               ident=np.eye(128, dtype=f32))

    sd_m_flat = sd_m_np.reshape(VM * 3, 10)
    tmpl_m_flat = tmpl_m.reshape(VM * 3)

    in_maps = []
    vid_all = np.full((NCORES, ROWS), -1, np.int64)

    for c in range(NCORES):
        p_ids = plain_sv[pl_sp[c]:pl_sp[c + 1]]
        h_pos = np.arange(hd_sp[c], hd_sp[c + 1])
        h_sv, h_fv = head_sv[h_pos], head_ix[h_pos]
        l_pos = np.arange(hl_sp[c], hl_sp[c + 1])
        r_pos = l_pos                                         # same split for R
        l_sv, r_sv = s2l[l_pos], s2r[r_pos]

        vid = np.full(ROWS, -1, np.int64)
        vid[:len(p_ids)] = p_ids
        vid[N_PLAIN:N_PLAIN + len(h_sv)] = h_sv
        vid[N_PLAIN + N_HEAD:N_PLAIN + N_HEAD + len(l_sv)] = l_sv
        vid[N_PLAIN + N_HEAD + N_HL:N_PLAIN + N_HEAD + N_HL + len(r_sv)] = r_sv
        vid_all[c] = vid
        vok = vid >= 0
        vc = np.where(vok, vid, 0)

        # smplx shapedirs slab [NCH, 128(p=l), (c, lk, v)]
        sdp = np.zeros((ROWS, 3, 384), f32)
        sdp[:, :, :NL] = np.where(vok[:, None, None], sd_s_np[vc], 0.0)
        sdp[:, :, NL] = np.where(vok[:, None], tmpl_s[vc], 0.0)
        slab = sdp.reshape(NCH, 128, 3, 3, 128).transpose(0, 4, 2, 3, 1)
        sd_s = np.ascontiguousarray(slab).reshape(NCH, 128, 1152)

        colv = vc[:, None] * 3 + np.arange(3)[None, :]
        pdv = pd_s_np[:PD_S_K][:, colv]
        pdv = np.where(vok[None, :, None], pdv, 0.0)
        pdv = pdv.reshape(PD_S_K, NCH, 128, 3).transpose(1, 0, 3, 2)
        pd_s_a = np.ascontiguousarray(pdv[:, :128]).reshape(NCH, 128, 384)
        pd_s_b = np.ascontiguousarray(pdv[:, 128:]).reshape(NCH, PD_S_K - 128, 384)

        jr_s = np.ascontiguousarray(
            np.where(vok[:, None], jr_s_np[:, vc].T, 0.0).reshape(NCH, 128, 55))
        w_s = np.ascontiguousarray(
            np.where(vok[:, None], w_s_np[vc], 0.0)
            .reshape(NCH, 128, 55).transpose(0, 2, 1))

        # flame: 5 even + 3 gathered chunks
        fe = _pad_ids(np.arange(fe_sp[c], fe_sp[c + 1]), NFE_CH * 128)
        fg = _pad_ids(h_fv, N_HEAD)
        f_rows = np.concatenate([fe, fg])
        fok = f_rows >= 0
        fc = np.where(fok, f_rows, 0)
        sdfp = np.zeros((len(f_rows), 3, 384), f32)
        sdfp[:, :, :NL] = np.where(fok[:, None, None], sd_f_np[fc], 0.0)
        sdfp[:, :, NL] = np.where(fok[:, None], tmpl_f[fc], 0.0)
        slab = sdfp.reshape(-1, 128, 3, 3, 128).transpose(0, 4, 2, 3, 1)
        sd_f = np.ascontiguousarray(slab).reshape(-1, 128, 1152)

        jr_f = np.ascontiguousarray(
            np.where(fok[:NFE_CH * 128, None], jr_f_np[:, fc[:NFE_CH * 128]].T, 0.0)
            .reshape(NFE_CH, 128, 5))

        fgc, fgok = fc[NFE_CH * 128:], fok[NFE_CH * 128:]
        colf = fgc[:, None] * 3 + np.arange(3)[None, :]
        pdfv = pd_f_np[9:36][:, colf]
        pdfv = np.where(fgok[None, :, None], pdfv, 0.0)
        pdfv = pdfv.reshape(PD_F_K, 3, 128, 3).transpose(1, 0, 3, 2)
        pd_f = np.ascontiguousarray(pdfv).reshape(3, PD_F_K, 384)

        wre = np.zeros((3, 11, 128), f32)
        for k in range(3):
            rows, ok = fgc[k * 128:(k + 1) * 128], fgok[k * 128:(k + 1) * 128]
            wre[k, :5] = np.where(ok[None, :], w_f_np[rows].T, 0.0)
            wre[k, 5:8] = np.where(ok[None, :], re_np[rows].T, 0.0)
            wre[k, 8:11] = np.where(ok[None, :], le_np[rows].T, 0.0)

        # mano hands + J shard
        m_rows = np.stack([_pad_ids(l_pos, 128), _pad_ids(r_pos, 128)])
        mok = m_rows >= 0
        mc = np.where(mok, m_rows, 0)
        sd_m = np.zeros((2, 11, 384), f32)
        pd_m_a = np.zeros((2, 128, 384), f32)
        pd_m_b = np.zeros((2, PD_M_K - 128, 384), f32)
        w_m = np.zeros((2, 16, 128), f32)
        for h in range(2):
            sdm = np.where(mok[h][:, None, None], sd_m_np[mc[h]], 0.0)
            sd_m[h, :10] = sdm.transpose(2, 1, 0).reshape(10, 384)
            sd_m[h, 10] = np.where(mok[h][:, None], tmpl_m[mc[h]], 0.0).T.reshape(384)
            colm = mc[h][:, None] * 3 + np.arange(3)[None, :]
            pdm = pd_m_np[:, colm]
            pdm = np.where(mok[h][None, :, None], pdm, 0.0).transpose(0, 2, 1)
            pd_m_a[h] = pdm[:128].reshape(128, 384)
            pd_m_b[h] = pdm[128:].reshape(PD_M_K - 128, 384)
            w_m[h] = np.where(mok[h][None, :], w_m_np[mc[h]].T, 0.0)

        mj = _pad_ids(np.arange(mj_sp[c], mj_sp[c + 1]), NMJ_CH * 128)
        mjok = mj >= 0
        mjc = np.where(mjok, mj, 0)
        sd_mj = np.concatenate(
            [np.where(mjok[:, None], sd_m_flat[mjc], 0.0),
             np.where(mjok, tmpl_m_flat[mjc], 0.0)[:, None]], 1)
        sd_mj = np.ascontiguousarray(
            sd_mj.reshape(NMJ_CH, 128, 11).transpose(0, 2, 1))
        jreg_m = np.zeros((NMJ_CH * 128, 48), f32)
        vv, cc3 = mjc // 3, mjc % 3
        jj = np.arange(16)
        jreg_m[np.arange(NMJ_CH * 128)[:, None], jj[None, :] * 3 + cc3[:, None]] = \
            np.where(mjok[:, None], jr_m_np[:, vv].T, 0.0)
        jreg_m = jreg_m.reshape(NMJ_CH, 128, 48)

        m = dict(rep)
        m.update(sd_s=sd_s, pd_s_a=pd_s_a, pd_s_b=pd_s_b, jr_s=jr_s, w_s=w_s,
                 sd_f=sd_f, jr_f=jr_f, pd_f=pd_f, wre_f=wre,
                 sd_m=sd_m, pd_m_a=pd_m_a, pd_m_b=pd_m_b, w_m=w_m,
                 sd_mj=sd_mj, jreg_m=jreg_m)
        out = {}
        for k, v in m.items():
            if k in BF16_INPUTS:
                out[k] = np.ascontiguousarray(v.astype(BF16NP))
            else:
                out[k] = np.ascontiguousarray(v, f32)
        in_maps.append(out)

    return in_maps, vid_all


# ================================================================ device IR

def _build_nc():
    nc = bacc.Bacc("TRN2", target_bir_lowering=False, debug=False,
                   num_devices=NCORES)
    di = {}

    def din(name, shape):
        dt = BF16 if name in BF16_INPUTS else F32
        di[name] = nc.dram_tensor(name, list(shape), dt, kind="ExternalInput").ap()

    din("aa", (B, 165)); din("aux", (B, 14))
    din("betaT_s", (3, 128, 128)); din("betaT_f", (3, 128, 128))
    din("joffT", (55, 384))
    din("mrelT_all", (92, 92))
    din("ancT_s", (55, 55)); din("ancT_f", (5, 5)); din("ancT_m", (16, 16))
    din("betam", (11, 1)); din("ident", (128, 128))
    din("sd_s", (NCH, 128, 1152)); din("pd_s_a", (NCH, 128, 384))
    din("pd_s_b", (NCH, PD_S_K - 128, 384))
    din("jr_s", (NCH, 128, 55)); din("w_s", (NCH, 55, 128))
    din("sd_f", (8, 128, 1152)); din("jr_f", (NFE_CH, 128, 5))
    din("pd_f", (3, PD_F_K, 384)); din("wre_f", (3, 11, 128))
    din("sd_m", (2, 11, 384)); din("pd_m_a", (2, 128, 384))
    din("pd_m_b", (2, PD_M_K - 128, 384)); din("w_m", (2, 16, 128))
    din("sd_mj", (NMJ_CH, 11, 128)); din("jreg_m", (NMJ_CH, 128, 48))

    out_d = nc.dram_tensor("out", [ROWS, 384], F32, kind="ExternalOutput").ap()
    dbg_d = None
    if DEBUG:
        dbg_d = nc.dram_tensor("dbg", [128, 4096], F32, kind="ExternalOutput").ap()

    with tile.TileContext(nc) as tc:
        _emit(nc, tc, di, out_d, dbg_d)
    nc.compile()
    return nc


def _emit(nc, tc, di, out_d, dbg_d=None):
    levels = _fk_levels(_fk_forest())
    es = ExitStack()
    persist = es.enter_context(tc.tile_pool(name="persist", bufs=1))
    slabs = es.enter_context(tc.tile_pool(name="slabs", bufs=3))
    acc_cm = tc.tile_pool(name="acc", bufs=4, space="PSUM")
    acc = acc_cm.__enter__()
    acct_cm = tc.tile_pool(name="acct", bufs=2, space="PSUM")
    acct = acct_cm.__enter__()
    jpool_cm = tc.tile_pool(name="jpool", bufs=1, space="PSUM")
    jpool = jpool_cm.__enter__()
    dram = es.enter_context(tc.tile_pool(name="dram", bufs=1, space="DRAM"))

    V, S, G, T, DMA = nc.vector, nc.scalar, nc.gpsimd, nc.tensor, nc.sync

    def ptile(shape, name):
        return persist.tile(list(shape), F32, tag=name, name=name)

    # ---------------- constants / staged inputs --------------------------

    aa = ptile((B, 165), "aa"); DMA.dma_start(aa[:], di["aa"][:])
    aux = ptile((B, 14), "aux"); DMA.dma_start(aux[:], di["aux"][:])

    betaT_s = persist.tile([128, 384], BF16, tag="betaT_s", name="betaT_s")
    betaT_f = persist.tile([128, 384], BF16, tag="betaT_f", name="betaT_f")
    for lk in range(3):
        DMA.dma_start(betaT_s[:, lk * 128:(lk + 1) * 128], di["betaT_s"][lk])
    betam = persist.tile([11, 1], BF16, tag="betam", name="betam"); DMA.dma_start(betam[:], di["betam"][:])

    # preloaded small per-chunk tensors (one DMA each, persist in SBUF)
    jr_all = persist.tile([128, NCH * 55], BF16, tag="jr_all", name="jr_all")
    DMA.dma_start(jr_all[:].rearrange("p (n k) -> p n k", k=55),
                  di["jr_s"][:].rearrange("n p k -> p n k"))
    for lk in range(3):
        DMA.dma_start(betaT_f[:, lk * 128:(lk + 1) * 128], di["betaT_f"][lk])
    ident = ptile((128, 128), "ident")
    DMA.dma_start(ident[:], di["ident"][:])
    joffT = ptile((55, 384), "joffT"); DMA.dma_start(joffT[:], di["joffT"][:])
    mrelT_all = ptile((92, 92), "mrelT_all"); DMA.dma_start(mrelT_all[:], di["mrelT_all"][:])
    jrf_all = persist.tile([128, NFE_CH * 5], BF16, tag="jrf_all", name="jrf_all")
    DMA.dma_start(jrf_all[:].rearrange("p (n k) -> p n k", k=5),
                  di["jr_f"][:].rearrange("n p k -> p n k"))
    jrm_all = persist.tile([128, NMJ_CH * 48], BF16, tag="jrm_all", name="jrm_all")
    DMA.dma_start(jrm_all[:].rearrange("p (n k) -> p n k", k=48),
                  di["jreg_m"][:].rearrange("n p k -> p n k"))
    sdmj_all = persist.tile([11, NMJ_CH * 128], BF16, tag="sdmj_all", name="sdmj_all")
    DMA.dma_start(sdmj_all[:].rearrange("p (n k) -> p n k", k=128),
                  di["sd_mj"][:].rearrange("n p k -> p n k"))
    w_all = persist.tile([55, NCH * 128], BF16, tag="w_all", name="w_all")
    DMA.dma_start(w_all[:].rearrange("p (n k) -> p n k", k=128),
                  di["w_s"][:].rearrange("n p k -> p n k"))
    wre_all = persist.tile([11, 3 * 128], BF16, tag="wre_all", name="wre_all")
    DMA.dma_start(wre_all[:].rearrange("p (n k) -> p n k", k=128),
                  di["wre_f"][:].rearrange("n p k -> p n k"))
    wm_all = persist.tile([16, 2 * 128], BF16, tag="wm_all", name="wm_all")
    DMA.dma_start(wm_all[:].rearrange("p (n k) -> p n k", k=128),
                  di["w_m"][:].rearrange("n p k -> p n k"))
    sdm_all = persist.tile([11, 2 * 384], BF16, tag="sdm_all", name="sdm_all")
    DMA.dma_start(sdm_all[:].rearrange("p (n k) -> p n k", k=384),
                  di["sd_m"][:].rearrange("n p k -> p n k"))

    # early zero-fills (vector queue; keeps the gpsimd queue free for CC)
    jsb2 = ptile((21, 384), "jsb2")
    V.memset(jsb2[:], 0.0)
    rhs_f = persist.tile([11, 2304], BF16, tag="rhs_f", name="rhs_f")
    V.memset(rhs_f[:], 0.0)
    rot_all = ptile((B, NJ_ALL * 9), "rot_all")
    ra3 = rot_all[:].rearrange("p (j x) -> p j x", x=9)
    ra4 = rot_all[:].rearrange("p (j m n) -> p j m n", m=3, n=3)
    V.memset(rot_all[:], 0.0)
    V.memset(ra3[:, :, 0:9:4], 1.0)
    jall = ptile((96, 400), "jall")

    # ---------------- stage A: blend shapes + J partials ------------------
    jpt = jpool.tile([128, 512], F32, tag="jpsum", name="jpt")
    jps = jpt[:, 0:384]
    jpt2 = jpool.tile([128, 512], F32, tag="jpsum2", name="jpt2")
    jps_f = jpt2[:, 0:384]
    jps_m = jpt2[:, 384:385]

    vp_sbuf = [persist.tile([128, 384], BF16, tag=f"vp{i}", name=f"vp{i}")
               for i in range(NCH)]
    vpf_sbuf = [persist.tile([128, 384], BF16, tag=f"vpf{h}", name=f"vpf{h}")
                for h in range(3)]
    vpm_sbuf = [persist.tile([128, 384], BF16, tag=f"vpm{h}", name=f"vpm{h}")
                for h in range(2)]

    def sd_mms(pp, slab_t, betaT, last=True):
        for c3 in range(3):
            for lk in range(3):
                T.matmul(pp[:, c3 * 128:(c3 + 1) * 128],
                         slab_t[:, (c3 * 3 + lk) * 128:(c3 * 3 + lk + 1) * 128],
                         betaT[:, lk * 128:(lk + 1) * 128],
                         start=(lk == 0), stop=(lk == 2 and last))

    vsb = [persist.tile([128, 384], BF16, tag=f"vsb{i}", name=f"vsb{i}")
           for i in range(NCH)]
    vsf32 = {i: ptile((128, 384), f"vsf32{i}") for i in CH_PLAIN}

    # ---- A1: shape blend + J partials (everything the AllReduce needs) ----
    for i in range(NCH):
        sdt = slabs.tile((128, 1152), BF16, tag="sd_s")
        DMA.dma_start(sdt[:], di["sd_s"][i])
        pp = acc.tile([128, 384], F32, tag="vppsum", padded_shape=[128, 512])
        sd_mms(pp, sdt, betaT_s)
        S.copy(vsb[i][:], pp[:])
        if i in CH_PLAIN:
            V.tensor_copy(vsf32[i][:], pp[:])
        T.matmul(jps[0:55, :], jr_all[:, i * 55:(i + 1) * 55], vsb[i][:],
                 start=(i == 0), stop=(i == NCH - 1))

    # ---- AR1: smplx J AllReduce (launched before flame/mano A1) ----------
    jsb = ptile((55, 384), "jsb")
    S.copy(jsb[:], jps[0:55, :])
    ar_in1 = dram.tile([55, 384], F32, tag="ar_in1")
    ar_out1 = dram.tile([55, 384], F32, tag="ar_out1")
    DMA.dma_start(ar_in1[:], jsb[:])
    G.collective_compute("AllReduce", ALU.add,
                         replica_groups=[list(range(NCORES))],
                         ins=[ar_in1[:].opt()], outs=[ar_out1[:].opt()])
    arr = ptile((55, 384), "arr")
    G.dma_start(arr[:], ar_out1[:])

    for k in range(NFE_CH):
        sdt = slabs.tile((128, 1152), BF16, tag="sd_f")
        DMA.dma_start(sdt[:], di["sd_f"][k])
        pp = acc.tile([128, 384], F32, tag="vppsum", padded_shape=[128, 512])
        sd_mms(pp, sdt, betaT_f)
        vsf = slabs.tile((128, 384), BF16, tag="vsf")
        S.copy(vsf[:], pp[:])
        T.matmul(jps_f[0:5, 0:384], jrf_all[:, k * 5:(k + 1) * 5], vsf[:],
                 start=(k == 0), stop=(k == NFE_CH - 1))

    for k in range(NMJ_CH):
        pp = acc.tile([128, 384], F32, tag="vppsum", padded_shape=[128, 512])
        T.matmul(pp[:, 0:1], sdmj_all[:, k * 128:(k + 1) * 128], betam[:],
                 start=True, stop=True)
        vsm = slabs.tile((128, 1), BF16, tag="vsmj")
        S.copy(vsm[:], pp[:, 0:1])
        T.matmul(jps_m[0:48, 0:1], jrm_all[:, k * 48:(k + 1) * 48], vsm[:],
                 start=(k == 0), stop=(k == NMJ_CH - 1))

    # ---- AR2: flame + mano J AllReduce -----------------------------------
    S.copy(jsb2[0:5, :], jps_f[0:5, 0:384])
    jsb_m = ptile((48, 1), "jsb_m")
    S.copy(jsb_m[:], jps_m[0:48, 0:1])
    jpool_cm.__exit__(None, None, None)
    ar_in2 = dram.tile([21, 384], F32, tag="ar_in2")
    ar_out2 = dram.tile([21, 384], F32, tag="ar_out2")
    DMA.dma_start(ar_in2[:], jsb2[:])
    DMA.dma_start(ar_in2[5:21, 0:3], jsb_m[:])
    G.collective_compute("AllReduce", ALU.add,
                         replica_groups=[list(range(NCORES))],
                         ins=[ar_in2[:].opt()], outs=[ar_out2[:].opt()])
    G.dma_start(jall[55:60, 0:384], ar_out2[0:5, :])
    G.dma_start(jall[64:80, 384:387], ar_out2[5:21, 0:3])
    G.dma_start(jall[80:96, 384:387], ar_out2[5:21, 0:3])

    # ---- eyelid rows of rhs_f (early: only needs aux) --------------------
    epp = ptile((B, 2), "epp")
    V.tensor_mul(epp[:], aux[:, 3:5], aux[:, 0:1].broadcast_to([B, 2]))
    epT = persist.tile([2, 128], BF16, tag="epT", name="epT")
    ppe = acct.tile([128, 384], F32, tag="tpose", padded_shape=[128, 512])
    T.matmul(ppe[0:2, 0:128], epp[:, :], ident[:], is_transpose=True,
             start=True, stop=True)
    S.copy(epT[:], ppe[0:2, 0:128])
    for m3 in range(3):
        DMA.dma_start(rhs_f[5 + m3:6 + m3, (12 + m3) * 128:(13 + m3) * 128],
                      epT[1:2, :])
        DMA.dma_start(rhs_f[8 + m3:9 + m3, (15 + m3) * 128:(16 + m3) * 128],
                      epT[0:1, :])

    # ---------------- rodrigues (vector; overlaps A1 on tensor) -----------
    rot = ptile((B, NROT * 9), "rot")
    _rodrigues(nc, aa, rot, ptile)
    rot4 = rot[:].rearrange("p (j x) -> p j x", x=9)

    def pf_make(name, j0, n):
        t = ptile((B, n * 9), name)
        t9 = t[:].rearrange("p (j x) -> p j x", x=9)
        V.tensor_copy(t9, rot4[:, j0:j0 + n, :])
        V.tensor_scalar_add(t9[:, :, 0:9:4], t9[:, :, 0:9:4], -1.0)
        return t

    pf_s = pf_make("pf_s", 1, 21)
    pf_f = pf_make("pf_f", 22, 3)
    pf_m = [pf_make("pf_l", 25, 15), pf_make("pf_r", 40, 15)]

    def transpose_to(dst_ap, src_ap):
        pp = acct.tile([128, 384], F32, tag="tpose", padded_shape=[128, 512])
        k, n = src_ap.shape[0], src_ap.shape[1]
        T.matmul(pp[:n, :k], src_ap, ident[:k, :k], is_transpose=True,
                 start=True, stop=True)
        S.copy(dst_ap, pp[:n, :k])

    pfT_s_a = persist.tile([128, 128], BF16, tag="pfT_s_a", name="pfT_s_a")
    pfT_s_b = persist.tile([PD_S_K - 128, 128], BF16, tag="pfT_s_b", name="pfT_s_b")
    transpose_to(pfT_s_a[:], pf_s[:, 0:128])
    transpose_to(pfT_s_b[:], pf_s[:, 128:PD_S_K])
    pfT_f = persist.tile([PD_F_K, 128], BF16, tag="pfT_f", name="pfT_f")
    transpose_to(pfT_f[:], pf_f[:, :])
    pfT_m_a = [persist.tile([128, 128], BF16, tag="pfT_l_a", name="pfT_l_a"), persist.tile([128, 128], BF16, tag="pfT_r_a", name="pfT_r_a")]
    pfT_m_b = [persist.tile([PD_M_K - 128, 128], BF16, tag="pfT_l_b", name="pfT_l_b"),
               persist.tile([PD_M_K - 128, 128], BF16, tag="pfT_r_b", name="pfT_r_b")]
    for h in range(2):
        transpose_to(pfT_m_a[h][:], pf_m[h][:, 0:128])
        transpose_to(pfT_m_b[h][:], pf_m[h][:, 128:PD_M_K])

    # ---- world rotations (vector; independent of the AllReduce) ----------
    V.tensor_copy(ra3[:, 0:22, :], rot4[:, 0:22, :])
    V.tensor_copy(ra3[:, 57:60, :], rot4[:, 22:25, :])
    V.tensor_copy(ra3[:, 61:76, :], rot4[:, 25:40, :])
    V.tensor_copy(ra3[:, 77:92, :], rot4[:, 40:55, :])
    negid = persist.tile([55, 55], BF16, tag="negid", name="negid")
    V.tensor_scalar_mul(negid[:], ident[0:55, 0:55], -1.0)
    ones3 = persist.tile([3, 16], BF16, tag="ones3", name="ones3")
    V.memset(ones3[:], 1.0)

    Rw = ptile((B, NJ_ALL * 9), "Rw")
    Rw4 = Rw[:].rearrange("p (j m n) -> p j m n", m=3, n=3)
    fk_scr = ptile((B, 16 * 9), "fk_scr")

    def rw_mul(dst_sl, par_sl, loc_sl, n, par_bcast=False):
        dst = Rw4[:, dst_sl]
        par = Rw4[:, par_sl]
        if par_bcast:
            par = par.broadcast_to([B, n, 3, 3])
        loc = ra4[:, loc_sl]
        sc = fk_scr[:].rearrange("p (j m n) -> p j m n", m=3, n=3)[:, :n]
        for k in range(3):
            a_k = par[:, :, :, k:k + 1].broadcast_to([B, n, 3, 3])
            t_k = loc[:, :, k:k + 1, :].broadcast_to([B, n, 3, 3])
            if k == 0:
                V.tensor_mul(dst, a_k, t_k)
            else:
                V.tensor_mul(sc, a_k, t_k)
                V.tensor_add(dst, dst, sc)

    V.tensor_copy(Rw4[:, 0:1], ra4[:, 0:1])
    rw_mul(slice(1, 4), slice(0, 1), slice(1, 4), 3, par_bcast=True)
    rw_mul(slice(4, 7), slice(1, 4), slice(4, 7), 3)
    rw_mul(slice(7, 10), slice(4, 7), slice(7, 10), 3)
    rw_mul(slice(10, 13), slice(7, 10), slice(10, 13), 3)
    rw_mul(slice(13, 15), slice(9, 10), slice(13, 15), 2, par_bcast=True)
    rw_mul(slice(15, 18), slice(12, 15), slice(15, 18), 3)
    rw_mul(slice(18, 20), slice(16, 18), slice(18, 20), 2)
    V.tensor_copy(Rw4[:, 22:25], Rw4[:, 15:16].broadcast_to([B, 3, 3, 3]))
    rw_mul(slice(20, 22), slice(18, 20), slice(20, 22), 2)
    V.tensor_copy(Rw4[:, 25:40], Rw4[:, 20:21].broadcast_to([B, 15, 3, 3]))
    V.tensor_copy(Rw4[:, 40:55], Rw4[:, 21:22].broadcast_to([B, 15, 3, 3]))
    # flame roots/jaw/eyes + mano roots and level-1 (parents are identity)
    V.tensor_copy(Rw4[:, 55:61], ra4[:, 55:61])
    V.tensor_copy(Rw4[:, 76:77], ra4[:, 76:77])
    V.tensor_copy(Rw4[:, 61:74:3], ra4[:, 61:74:3])
    V.tensor_copy(Rw4[:, 77:90:3], ra4[:, 77:90:3])
    rw_mul(slice(62, 75, 3), slice(61, 74, 3), slice(62, 75, 3), 5)
    rw_mul(slice(78, 91, 3), slice(77, 90, 3), slice(78, 91, 3), 5)
    rw_mul(slice(63, 76, 3), slice(62, 75, 3), slice(63, 76, 3), 5)
    rw_mul(slice(79, 92, 3), slice(78, 91, 3), slice(79, 92, 3), 5)

    # ---- A2: posedirs + flame/mano v_posed (overlaps the AllReduce) ------
    for i in range(NCH):
        pda = slabs.tile((128, 384), BF16, tag="pd_s_a")
        pdb = slabs.tile((PD_S_K - 128, 384), BF16, tag="pd_s_b")
        DMA.dma_start(pda[:], di["pd_s_a"][i])
        DMA.dma_start(pdb[:], di["pd_s_b"][i])
        pq = acc.tile([128, 384], F32, tag="vppsum", padded_shape=[128, 512])
        for c3 in range(3):
            T.matmul(pq[:, c3 * 128:(c3 + 1) * 128],
                     pda[:, c3 * 128:(c3 + 1) * 128], pfT_s_a[:],
                     start=True, stop=False)
            T.matmul(pq[:, c3 * 128:(c3 + 1) * 128],
                     pdb[:, c3 * 128:(c3 + 1) * 128], pfT_s_b[:],
                     start=False, stop=True)
        if i in CH_PLAIN:
            V.tensor_add(vp_sbuf[i][:], vsf32[i][:], pq[:])
        else:
            S.copy(vp_sbuf[i][:], pq[:])

    for h in range(3):
        sdt = slabs.tile((128, 1152), BF16, tag="sd_f")
        DMA.dma_start(sdt[:], di["sd_f"][NFE_CH + h])
        pp = acc.tile([128, 384], F32, tag="vppsum", padded_shape=[128, 512])
        pdf = slabs.tile((PD_F_K, 384), BF16, tag="pd_f")
        DMA.dma_start(pdf[:], di["pd_f"][h])
        for c3 in range(3):
            for lk in range(3):
                T.matmul(pp[:, c3 * 128:(c3 + 1) * 128],
                         sdt[:, (c3 * 3 + lk) * 128:(c3 * 3 + lk + 1) * 128],
                         betaT_f[:, lk * 128:(lk + 1) * 128],
                         start=(lk == 0), stop=False)
            T.matmul(pp[:, c3 * 128:(c3 + 1) * 128],
                     pdf[:, c3 * 128:(c3 + 1) * 128], pfT_f[:],
                     start=False, stop=True)
        S.copy(vpf_sbuf[h][:], pp[:])

    for h in range(2):
        pps = acc.tile([128, 384], F32, tag="vppsum", padded_shape=[128, 512])
        for c3 in range(3):
            T.matmul(pps[:, c3:c3 + 1], sdm_all[:, h * 384 + c3 * 128:h * 384 + (c3 + 1) * 128],
                     betam[:], start=True, stop=True)
        vshm = ptile((128, 3), f"vshm{h}")
        S.copy(vshm[:], pps[:, 0:3])
        pda = slabs.tile((128, 384), BF16, tag="pd_m_a")
        pdb = slabs.tile((PD_M_K - 128, 384), BF16, tag="pd_m_b")
        DMA.dma_start(pda[:], di["pd_m_a"][h])
        DMA.dma_start(pdb[:], di["pd_m_b"][h])
        pq = acc.tile([128, 384], F32, tag="vppsum", padded_shape=[128, 512])
        for c3 in range(3):
            T.matmul(pq[:, c3 * 128:(c3 + 1) * 128],
                     pda[:, c3 * 128:(c3 + 1) * 128], pfT_m_a[h][:],
                     start=True, stop=False)
            T.matmul(pq[:, c3 * 128:(c3 + 1) * 128],
                     pdb[:, c3 * 128:(c3 + 1) * 128], pfT_m_b[h][:],
                     start=False, stop=True)
        vpm = vpm_sbuf[h]
        V.tensor_add(vpm[:].rearrange("p (c b) -> p c b", b=128),
                     pq[:].rearrange("p (c b) -> p c b", b=128),
                     vshm[:].unsqueeze(2).broadcast_to([128, 3, 128]))

    # ================= joints + A_rel assembly (post-AllReduce) ===========
    arr3 = arr  # smplx J sum from AR1
    V.tensor_add(jall[0:55, 0:384], arr3[:], joffT[:])
    # broadcast compact mano J into (c,b) layout on an aligned scratch tile,
    # then DMA into jall rows 60:92 (engine partition starts must be 32-aligned)
    jmtmp = ptile((32, 384), "jmtmp")
    V.tensor_copy(jmtmp[:].rearrange("p (c b) -> p c b", b=128),
                  jall[64:96, 384:387].unsqueeze(2).broadcast_to([32, 3, 128]))
    G.dma_start(jall[60:92, 0:384], jmtmp[:])

    # rel = mrel_all @ J (one fp32 matmul over the whole forest)
    ppr = acct.tile([128, 384], F32, tag="tpose", padded_shape=[128, 512])
    T.matmul(ppr[0:92, 0:384], mrelT_all[:], jall[0:92, 0:384],
             start=True, stop=True)
    rel_all = ptile((92, 384), "rel_all")
    S.copy(rel_all[:], ppr[0:92, 0:384])

    # batch-major J and rel:  jrb[:, 0:276] = J (c-major), [:, 280:556] = rel
    jrb = ptile((B, 560), "jrb")
    for c3 in range(3):
        ppj = acct.tile([128, 384], F32, tag="tpose", padded_shape=[128, 512])
        T.matmul(ppj[0:128, 0:92], jall[0:92, c3 * 128:(c3 + 1) * 128],
                 ident[0:92, 0:92], is_transpose=True, start=True, stop=True)
        T.matmul(ppj[0:128, 192:284], rel_all[:, c3 * 128:(c3 + 1) * 128],
                 ident[0:92, 0:92], is_transpose=True, start=True, stop=True)
        S.copy(jrb[:].rearrange("p (t x) -> p t x", x=280)[:, :, c3 * 92:(c3 + 1) * 92],
               ppj[:].rearrange("p (t x) -> p t x", x=192)[:, :, 0:92])

    jbv = jrb[:, 0:276].rearrange("p (c a) -> p c a", c=3)
    relv = jrb[:, 280:556].rearrange("p (c a) -> p c a", c=3)

    # q_a = R_a^T rel_a ; c_a = R^w_a q_a ; u_a = R^w_a J_a   (all joints)
    qv = ptile((B, 3 * NJ_ALL), "qv")
    cv = ptile((B, 3 * NJ_ALL), "cv")
    uv = ptile((B, 3 * NJ_ALL), "uv")
    scr3 = ptile((B, 3 * NJ_ALL), "scr3")
    q3 = qv[:].rearrange("p (a k) -> p a k", k=3)
    c3v = cv[:].rearrange("p (a k) -> p a k", k=3)
    u3 = uv[:].rearrange("p (a k) -> p a k", k=3)
    s3 = scr3[:].rearrange("p (a k) -> p a k", k=3)
    for m in range(3):
        rm = relv[:, m, :].unsqueeze(2).broadcast_to([B, NJ_ALL, 3])
        if m == 0:
            V.tensor_mul(q3, ra4[:, :, 0, :], rm)
        else:
            V.tensor_mul(s3, ra4[:, :, m, :], rm)
            V.tensor_add(q3, q3, s3)
    for k in range(3):
        qk = q3[:, :, k].unsqueeze(2).broadcast_to([B, NJ_ALL, 3])
        if k == 0:
            V.tensor_mul(c3v, Rw4[:, :, :, 0], qk)
        else:
            V.tensor_mul(s3, Rw4[:, :, :, k], qk)
            V.tensor_add(c3v, c3v, s3)
    gscr = ptile((B, 3 * NJ_ALL), "gscr")
    g3 = gscr[:].rearrange("p (a k) -> p a k", k=3)
    for k in range(3):
        jk = jbv[:, k, :].unsqueeze(2).broadcast_to([B, NJ_ALL, 3])
        if k == 0:
            G.tensor_mul(u3, Rw4[:, :, :, 0], jk)
        else:
            G.tensor_mul(g3, Rw4[:, :, :, k], jk)
            G.tensor_add(u3, u3, g3)

    # ---- scale / mirror folding (batch-major) ----------------------------
    V.tensor_scalar_mul(Rw[:, 495:540], Rw[:, 495:540], aux[:, 0:1])
    V.tensor_scalar_mul(cv[:, 165:180], cv[:, 165:180], aux[:, 0:1])
    V.tensor_scalar_mul(uv[:, 165:180], uv[:, 165:180], aux[:, 0:1])
    negls = ptile((B, 1), "negls")
    V.tensor_scalar_mul(negls[:], aux[:, 1:2], -1.0)
    V.tensor_scalar_mul(Rw4[:, 60:76, 0, :], Rw4[:, 60:76, 0, :], negls[:, 0:1])
    V.tensor_scalar_mul(Rw4[:, 60:76, 1:3, :], Rw4[:, 60:76, 1:3, :], aux[:, 1:2])
    V.tensor_scalar_mul(c3v[:, 60:76, 0], c3v[:, 60:76, 0], negls[:, 0:1])
    V.tensor_scalar_mul(c3v[:, 60:76, 1:3], c3v[:, 60:76, 1:3], aux[:, 1:2])
    V.tensor_scalar_mul(u3[:, 60:76, 0], u3[:, 60:76, 0], negls[:, 0:1])
    V.tensor_scalar_mul(u3[:, 60:76, 1:3], u3[:, 60:76, 1:3], aux[:, 1:2])
    V.tensor_scalar_mul(Rw[:, 684:828], Rw[:, 684:828], aux[:, 2:3])
    V.tensor_scalar_mul(cv[:, 228:276], cv[:, 228:276], aux[:, 2:3])
    V.tensor_scalar_mul(uv[:, 228:276], uv[:, 228:276], aux[:, 2:3])

    # ---- per-batch bias vectors (head / left / right) --------------------
    bias9 = ptile((B, 9), "bias9")
    hm = ptile((B, 6), "hm")
    hl = ptile((B, 3), "hl")
    hr = ptile((B, 3), "hr")
    V.tensor_add(hm[:, 0:3], jbv[:, :, 23], jbv[:, :, 24])
    V.tensor_add(hm[:, 3:6], jbv[:, :, 58], jbv[:, :, 59])
    V.tensor_sub(bias9[:, 0:3], hm[:, 0:3], hm[:, 3:6])
    V.tensor_scalar_mul(bias9[:, 0:3], bias9[:, 0:3], 0.5)
    V.tensor_add(bias9[:, 0:3], bias9[:, 0:3], aux[:, 5:8])
    V.tensor_sub(hl[:], aux[:, 8:11], jbv[:, :, 60])
    V.tensor_sub(bias9[:, 3:4], jbv[:, 0:1, 20], hl[:, 0:1])
    V.tensor_add(bias9[:, 4:6], jbv[:, 1:3, 20], hl[:, 1:3])
    V.tensor_sub(hr[:], aux[:, 11:14], jbv[:, :, 60])
    V.tensor_add(bias9[:, 6:9], jbv[:, :, 21], hr[:])

    # bsT per group -> [1, 384] bias rhs rows (DMA reshapes [3,128]->[1,384])
    rbias = []
    for g in range(3):
        ppb = acct.tile([128, 384], F32, tag="tpose", padded_shape=[128, 512])
        T.matmul(ppb[0:3, 0:128], bias9[:, g * 3:(g + 1) * 3], ident[:],
                 is_transpose=True, start=True, stop=True)
        bst = persist.tile([3, 128], BF16, tag=f"bsT{g}", name=f"bsT{g}")
        S.copy(bst[:], ppb[0:3, 0:128])
        rb = persist.tile([1, 384], BF16, tag=f"rbias{g}", name=f"rbias{g}")
        G.dma_start(rb[:], bst[:])
        rbias.append(rb)

    # ---- rhs assembly ----------------------------------------------------
    rhs_s = persist.tile([55, 1536], BF16, tag="rhs_s", name="rhs_s")
    rhs_m = [persist.tile([16, 1536], BF16, tag="rhs_l", name="rhs_l"),
             persist.tile([16, 1536], BF16, tag="rhs_r", name="rhs_r")]
    groups = [(0, 55, rhs_s, di["ancT_s"], None),
              (55, 5, rhs_f, di["ancT_f"], 0),
              (60, 16, rhs_m[0], di["ancT_m"], 1),
              (76, 16, rhs_m[1], di["ancT_m"], 2)]
    ancT_t = {}
    for nm in ("ancT_s", "ancT_f", "ancT_m"):
        n = di[nm].shape[0]
        t = persist.tile([n, n], BF16, tag=nm, name=nm + "_t")
        DMA.dma_start(t[:], di[nm][:])
        ancT_t[nm] = t

    for (a0, ng, rhs_t, anc_d, bias_g) in groups:
        anc_t = ancT_t["ancT_s" if ng == 55 else ("ancT_f" if ng == 5 else "ancT_m")]
        # cT/uT via strided transposes:  cuT[:, 0:384]=c^T, [:, 384:768]=u^T
        cuT = persist.tile([ng, 768], BF16, tag=f"cuT{a0}", name=f"cuT{a0}")
        for m in range(3):
            ppc = acct.tile([128, 384], F32, tag="tpose", padded_shape=[128, 512])
            T.matmul(ppc[0:ng, 0:128],
                     cv[:, 3 * a0 + m:3 * (a0 + ng - 1) + m + 1:3],
                     ident[:], is_transpose=True, start=True, stop=True)
            T.matmul(ppc[0:ng, 192:320],
                     uv[:, 3 * a0 + m:3 * (a0 + ng - 1) + m + 1:3],
                     ident[:], is_transpose=True, start=True, stop=True)
            S.copy(cuT[0:ng].rearrange("p (t x) -> p t x", x=384)[:, :, m * 128:(m + 1) * 128],
                   ppc[0:ng].rearrange("p (t x) -> p t x", x=192)[:, :, 0:128])
        # translations: Anc @ c - u (+ bias) -> rhs cols 1152:1536
        ppt = acct.tile([128, 384], F32, tag="tpose", padded_shape=[128, 512])
        T.matmul(ppt[0:ng, 0:384], anc_t[:], cuT[0:ng, 0:384],
                 start=True, stop=False)
        T.matmul(ppt[0:ng, 0:384], negid[0:ng, 0:ng], cuT[0:ng, 384:768],
                 start=False, stop=(bias_g is None))
        if bias_g is not None:
            T.matmul(ppt[0:ng, 0:384], ones3[0:1, 0:ng], rbias[bias_g][:],
                     start=False, stop=True)
        S.copy(rhs_t[0:ng, 1152:1536], ppt[0:ng, 0:384])
        # rotations: R^w columns -> rhs cols n*384 + m*128
        for n4 in range(3):
            ppn = acct.tile([128, 384], F32, tag="tpose", padded_shape=[128, 512])
            for m in range(3):
                T.matmul(ppn[0:ng, m * 128:(m + 1) * 128],
                         Rw[:, 9 * a0 + m * 3 + n4:9 * (a0 + ng - 1) + m * 3 + n4 + 1:9],
                         ident[:], is_transpose=True, start=True, stop=True)
            S.copy(rhs_t[0:ng, n4 * 384:(n4 + 1) * 384], ppn[0:ng, 0:384])

    acct_cm.__exit__(None, None, None)
    acc_cm.__exit__(None, None, None)
    big_cm = tc.tile_pool(name="big", bufs=2, space="PSUM")
    big = big_cm.__enter__()

    # ---------------- skinning per chunk (bf16 DVE fast-path) -------------
    def t_apply(dst_ap, tp_bf, x_sbuf, scratch):
        """dst = sum_{n<3} T'[n]*x_n + T'[3]; bf16 SBUF operands."""
        d3 = dst_ap.rearrange("p (m b) -> p m b", b=128)
        x3 = x_sbuf[:].rearrange("p (c b) -> p c b", b=128)
        tp = tp_bf[:].rearrange("p (n m b) -> p n m b", m=3, b=128)
        sc = scratch[:].rearrange("p (n m b) -> p n m b", m=3, b=128)
        V.tensor_mul(sc[:, 0:3], tp[:, 0:3],
                     x3[:].unsqueeze(2).broadcast_to([128, 3, 3, 128]))
        V.tensor_add(sc[:, 0], sc[:, 0], sc[:, 1])
        V.tensor_add(sc[:, 0], sc[:, 0], tp[:, 3])
        V.tensor_add(d3, sc[:, 0], sc[:, 2])

    scr_t = [persist.tile([128, 1152], BF16, tag=f"scr{i}", name=f"scr{i}")
             for i in range(4)]

    for i in range(NCH):
        if CH_HEAD0 <= i < CH_HEAD0 + 3:
            h = i - CH_HEAD0
            hv = slabs.tile((128, 384), BF16, tag="hv", bufs=2, name="hv")
            wt = wre_all[:, h * 128:(h + 1) * 128]
            tp1 = big.tile([128, 1536], F32, tag="bigp")
            for g, w in ((0, 512), (1, 512), (2, 128)):
                T.matmul(tp1[:, g * 512:g * 512 + w], wt,
                         rhs_f[:, g * 512:g * 512 + w], start=True, stop=True)
            tp2 = big.tile([128, 1536], F32, tag="bigp")
            for g, w in ((0, 512), (1, 512), (2, 128)):
                T.matmul(tp2[:, g * 512:g * 512 + w], wt,
                         rhs_f[:, 1152 + g * 512:1152 + g * 512 + w],
                         start=True, stop=True)
            tb1 = slabs.tile((128, 1152), BF16, tag="tpb1", bufs=2, name="tb1")
            S.copy(tb1[:], tp1[:, 0:1152])
            tb2 = slabs.tile((128, 1152), BF16, tag="tpb2", bufs=2, name="tb2")
            V.tensor_copy(tb2[:], tp2[:, 0:1152])
            d3 = hv[:].rearrange("p (m b) -> p m b", b=128)
            x3 = vpf_sbuf[h][:].rearrange("p (c b) -> p c b", b=128)
            t1 = tb1[:].rearrange("p (n m b) -> p n m b", m=3, b=128)
            t2 = tb2[:].rearrange("p (n m b) -> p n m b", m=3, b=128)
            sc = scr_t[i % 4][:].rearrange("p (n m b) -> p n m b", m=3, b=128)
            V.tensor_mul(sc[:, 0:3], t1[:, 0:3],
                         x3[:].unsqueeze(2).broadcast_to([128, 3, 3, 128]))
            V.tensor_add(sc[:, 0], sc[:, 0], sc[:, 1])
            V.tensor_add(sc[:, 0], sc[:, 0], sc[:, 2])
            V.tensor_add(sc[:, 1], t2[:, 0], t2[:, 1])
            V.tensor_add(sc[:, 1], sc[:, 1], t2[:, 2])
            V.tensor_add(d3, sc[:, 0], sc[:, 1])
            V.tensor_add(vp_sbuf[i][:], vp_sbuf[i][:], hv[:])
        elif i in (CH_HL, CH_HR):
            h = i - CH_HL
            hv = slabs.tile((128, 384), BF16, tag="hv", bufs=2, name="hv")
            wt = wm_all[:, h * 128:(h + 1) * 128]
            tpm = big.tile([128, 1536], F32, tag="bigp")
            for g in range(3):
                T.matmul(tpm[:, g * 512:(g + 1) * 512], wt,
                         rhs_m[h][:, g * 512:(g + 1) * 512], start=True, stop=True)
            tbm = slabs.tile((128, 1536), BF16, tag="tpbm", bufs=2, name="tbm")
            S.copy(tbm[:], tpm[:])
            t_apply(hv[:], tbm, vpm_sbuf[h], scr_t[i % 4])
            V.tensor_add(vp_sbuf[i][:], vp_sbuf[i][:], hv[:])

        wt = w_all[:, i * 128:(i + 1) * 128]
        tps = big.tile([128, 1536], F32, tag="bigp")
        for g in range(3):
            T.matmul(tps[:, g * 512:(g + 1) * 512], wt,
                     rhs_s[:, g * 512:(g + 1) * 512], start=True, stop=True)
        tbs = slabs.tile((128, 1536), BF16, tag="tpbs", bufs=3, name="tbs")
        S.copy(tbs[:], tps[:])
        ot = slabs.tile((128, 384), F32, tag="outt", bufs=3, name="ot")
        t_apply(ot[:], tbs, vp_sbuf[i], scr_t[i % 4])
        DMA.dma_start(out_d[i * 128:(i + 1) * 128, :], ot[:])

    big_cm.__exit__(None, None, None)
    es.close()


def _rodrigues(nc, aa, rot, ptile):
    V, S = nc.vector, nc.scalar
    J = NROT
    aa3 = aa[:].rearrange("p (j k) -> p j k", k=3)
    sq = ptile((B, J), "rg_sq")
    tmp = ptile((B, J), "rg_tmp")
    V.tensor_mul(sq[:], aa3[:, :, 0], aa3[:, :, 0])
    V.tensor_mul(tmp[:], aa3[:, :, 1], aa3[:, :, 1])
    V.tensor_add(sq[:], sq[:], tmp[:])
    V.tensor_mul(tmp[:], aa3[:, :, 2], aa3[:, :, 2])
    V.tensor_add(sq[:], sq[:], tmp[:])
    eps_t = ptile((B, 1), "rg_eps")
    nc.gpsimd.memset(eps_t[:], 1e-8)
    hpi_t = ptile((B, 1), "rg_hpi")
    nc.gpsimd.memset(hpi_t[:], float(np.pi / 2))
    zero_t = ptile((B, 1), "rg_zero")
    nc.gpsimd.memset(zero_t[:], 0.0)
    ang = ptile((B, J), "rg_ang")
    S.activation(ang[:], sq[:], AF.Sqrt, bias=eps_t[:])
    inv = ptile((B, J), "rg_inv")
    V.reciprocal(inv[:], ang[:])
    sn = ptile((B, J), "rg_sin")
    co = ptile((B, J), "rg_cos")
    S.activation(sn[:], ang[:], AF.Sin, bias=zero_t[:])
    S.activation(co[:], ang[:], AF.Sin, bias=hpi_t[:])
    nv = ptile((B, 3 * J), "rg_n")
    n3 = nv[:].rearrange("p (j k) -> p j k", k=3)
    V.tensor_mul(n3, aa3, inv[:].unsqueeze(2).broadcast_to([B, J, 3]))
    u = ptile((B, J), "rg_u")
    V.tensor_scalar(u[:], co[:], -1.0, 1.0, ALU.mult, ALU.add)
    un = ptile((B, 3 * J), "rg_un")
    un3 = un[:].rearrange("p (j k) -> p j k", k=3)
    V.tensor_mul(un3, n3, u[:].unsqueeze(2).broadcast_to([B, J, 3]))
    q = ptile((B, 3 * J), "rg_q")
    q3 = q[:].rearrange("p (j k) -> p j k", k=3)
    V.tensor_mul(q3, un3, n3)
    d = ptile((B, J), "rg_d")
    V.tensor_add(d[:], q3[:, :, 0], q3[:, :, 1])
    V.tensor_add(d[:], d[:], q3[:, :, 2])
    dd = ptile((B, J), "rg_dd")
    V.tensor_scalar(dd[:], d[:], -1.0, 1.0, ALU.mult, ALU.add)
    snv = ptile((B, 3 * J), "rg_snv")
    s3 = snv[:].rearrange("p (j k) -> p j k", k=3)
    V.tensor_mul(s3, n3, sn[:].unsqueeze(2).broadcast_to([B, J, 3]))
    r4 = rot[:].rearrange("p (j m n) -> p j m n", m=3, n=3)
    for m in range(3):
        V.tensor_add(r4[:, :, m, m], q3[:, :, m], dd[:])
    p = ptile((B, J), "rg_p")
    V.tensor_mul(p[:], un3[:, :, 0], n3[:, :, 1])
    V.tensor_sub(r4[:, :, 0, 1], p[:], s3[:, :, 2])
    V.tensor_add(r4[:, :, 1, 0], p[:], s3[:, :, 2])
    V.tensor_mul(p[:], un3[:, :, 0], n3[:, :, 2])
    V.tensor_add(r4[:, :, 0, 2], p[:], s3[:, :, 1])
    V.tensor_sub(r4[:, :, 2, 0], p[:], s3[:, :, 1])
    V.tensor_mul(p[:], un3[:, :, 1], n3[:, :, 2])
    V.tensor_sub(r4[:, :, 1, 2], p[:], s3[:, :, 0])
    V.tensor_add(r4[:, :, 2, 1], p[:], s3[:, :, 0])


# ================================================================ entry

_CACHED = {}
DEBUG = False


def _get_nc():
    if "nc" not in _CACHED:
        _CACHED["nc"] = _build_nc()
    return _CACHED["nc"]


PROFILE = False
TRACE_DIR = None


def kernel(**inputs):
    in_maps, vid_all = _host_prep(inputs)
    nc = _get_nc()
    kw = {}
    if PROFILE and TRACE_DIR:
        kw["tmpdir"] = TRACE_DIR
    res = run_bass_kernel_spmd(nc, in_maps, core_ids=list(range(NCORES)),
                               trace=PROFILE, **kw)
    _CACHED["last_res"] = res
    out = np.zeros((B, VS, 3), np.float32)
    for c in range(NCORES):
        o = np.asarray(res.results[c]["out"]).reshape(ROWS, 3, B)
        vok = vid_all[c] >= 0
        out[:, vid_all[c][vok], :] = o[vok].transpose(2, 0, 1)
    return out



# revision 30
# speedup vs baseline: 1.1871x; 1.1521x over previous
"""EHM (SMPLX body + FLAME head + MANO hands) Bass kernel for 8 TRN2 NeuronCores.

Sharding: VERTEX sharding — model weights (shapedirs/posedirs/regressors/lbs
weights, ~130MB) dominate HBM traffic, so each core owns 1/8 of the SMPLX
vertices (plus the FLAME/MANO vertices its SMPLX rows stitch in) and computes
ALL B=128 batch elements for its shard.  The only cross-core dependency is the
joint regression J = J_regressor @ v_shaped -> one [76, 384] AllReduce of
partial joint sums.  FK (92 joints) is replicated on every core on the vector
engine with batch on partitions (B == 128 == n_partitions).

Per-vertex data layout: [vertex(partition<=128), (c, b)] with c-major free dim
(col = c*128 + b).  Batch-staged data (poses, FK, A matrices): [b(part), free].
"""

import sys

sys.path.insert(0, "/opt/trn_rl_repo")

from contextlib import ExitStack

import numpy as np
import ml_dtypes

BF16NP = ml_dtypes.bfloat16

import concourse.bass as bass
import concourse.bacc as bacc
import concourse.tile as tile
import concourse.mybir as mybir
from concourse.bass_utils import run_bass_kernel_spmd

F32 = mybir.dt.float32
BF16 = mybir.dt.bfloat16
AF = mybir.ActivationFunctionType
ALU = mybir.AluOpType

# ---------------------------------------------------------------- constants
B = 128
VS, VF, VM = 10475, 5023, 778
NL = 350
NCORES = 8

SMPLX_PARENTS = np.array([-1,0,0,0,1,2,3,4,5,6,7,8,9,9,9,12,13,14,16,17,18,19,
                          15,15,15,20,25,26,20,28,29,20,31,32,20,34,35,20,37,38,
                          21,40,41,21,43,44,21,46,47,21,49,50,21,52,53])
FLAME_PARENTS = np.array([-1,0,1,1,1])
MANO_PARENTS = np.array([-1,0,1,2,0,4,5,0,7,8,0,10,11,0,13,14])

N_PLAIN, N_HEAD, N_HL, N_HR = 768, 384, 128, 128
ROWS = N_PLAIN + N_HEAD + N_HL + N_HR        # 1408
NCH = ROWS // 128                            # 11
CH_PLAIN = set(range(0, 6))
CH_HEAD0 = 6                                 # chunks 6,7,8 head; 9 L; 10 R
CH_HL, CH_HR = 9, 10

NFE_CH = 5
NMJ_CH = 3
PD_S_K = 189
PD_F_K = 27
PD_M_K = 135

NJ_ALL = 92
OFF_S, OFF_F, OFF_L, OFF_R = 0, 55, 60, 76
NROT = 55
ROT_S0, ROT_F0, ROT_L0, ROT_R0 = 0, 22, 25, 40

BF16_INPUTS = {"w_s", "wre_f", "w_m", "ancT_s", "ancT_f", "ancT_m",
               "sd_s", "pd_s_a", "pd_s_b", "jr_s", "sd_f", "jr_f", "pd_f",
               "sd_m", "pd_m_a", "pd_m_b", "sd_mj", "jreg_m",
               "betaT_s", "betaT_f", "betam"}


def _fk_forest():
    par = np.empty(NJ_ALL, np.int64)
    par[OFF_S:OFF_S + 55] = SMPLX_PARENTS
    par[OFF_F:OFF_F + 5] = np.where(FLAME_PARENTS < 0, -1, FLAME_PARENTS + OFF_F)
    par[OFF_L:OFF_L + 16] = np.where(MANO_PARENTS < 0, -1, MANO_PARENTS + OFF_L)
    par[OFF_R:OFF_R + 16] = np.where(MANO_PARENTS < 0, -1, MANO_PARENTS + OFF_R)
    return par


def _fk_levels(par):
    depth = np.zeros(NJ_ALL, np.int64)
    for j in range(NJ_ALL):
        if par[j] >= 0:
            depth[j] = depth[par[j]] + 1
    levels = []
    for d in range(1, int(depth.max()) + 1):
        js = np.nonzero(depth == d)[0]
        runs, i = [], 0
        while i < len(js):
            j0, p0 = int(js[i]), int(par[js[i]])
            if i + 1 < len(js):
                ds = int(js[i + 1]) - j0
                ps = int(par[js[i + 1]]) - p0
            else:
                ds, ps = 1, 0
            n = 1
            while (i + n < len(js) and int(js[i + n]) == j0 + n * ds
                   and int(par[js[i + n]]) == p0 + n * ps):
                n += 1
            if n == 1:
                ds, ps = 1, 0
            runs.append((j0, ds, n, p0, ps))
            i += n
        levels.append(runs)
    return levels


# ================================================================ host prep

def _split_sizes(total, parts):
    q, r = divmod(total, parts)
    return [q + (1 if i < r else 0) for i in range(parts)]


def _pad_ids(ids, n):
    out = np.full(n, -1, np.int64)
    out[:len(ids)] = ids
    return out


def _host_prep(inp):
    f32 = np.float32
    s2f = np.asarray(inp["smplx2flame_ind"])
    head_ix = np.asarray(inp["head_index"])
    s2l = np.asarray(inp["smplx2mano_left"])
    s2r = np.asarray(inp["smplx2mano_right"])

    head_sv = s2f[head_ix]
    special = np.zeros(VS, bool)
    special[head_sv] = True
    special[s2l] = True
    special[s2r] = True
    plain_sv = np.nonzero(~special)[0]

    pl_sp = np.cumsum([0] + _split_sizes(len(plain_sv), NCORES))
    hd_sp = np.cumsum([0] + _split_sizes(len(head_ix), NCORES))
    hl_sp = np.cumsum([0] + _split_sizes(VM, NCORES))
    fe_sp = np.cumsum([0] + _split_sizes(VF, NCORES))
    mj_sp = np.cumsum([0] + _split_sizes(VM * 3, NCORES))

    sd_s_np = np.asarray(inp["smplx_shapedirs"], f32)
    pd_s_np = np.asarray(inp["smplx_posedirs"], f32)
    jr_s_np = np.asarray(inp["smplx_J_regressor"], f32)
    w_s_np = np.asarray(inp["smplx_lbs_weights"], f32)
    tmpl_s = np.asarray(inp["smplx_v_template"], f32)
    sd_f_np = np.asarray(inp["flame_shapedirs"], f32)
    pd_f_np = np.asarray(inp["flame_posedirs"], f32)
    jr_f_np = np.asarray(inp["flame_J_regressor"], f32)
    w_f_np = np.asarray(inp["flame_lbs_weights"], f32)
    tmpl_f = np.asarray(inp["flame_v_template"], f32)
    re_np = np.asarray(inp["r_eyelid"], f32)
    le_np = np.asarray(inp["l_eyelid"], f32)
    sd_m_np = np.asarray(inp["mano_shapedirs"], f32)
    pd_m_np = np.asarray(inp["mano_posedirs"], f32)
    jr_m_np = np.asarray(inp["mano_J_regressor"], f32)
    w_m_np = np.asarray(inp["mano_lbs_weights"], f32)
    tmpl_m = np.asarray(inp["mano_v_template"], f32)

    aa = np.concatenate([
        np.asarray(inp["global_pose"], f32).reshape(B, 3),
        np.asarray(inp["body_pose"], f32).reshape(B, 63),
        np.asarray(inp["jaw_params"], f32).reshape(B, 3),
        np.asarray(inp["eye_pose"], f32).reshape(B, 6),
        np.asarray(inp["left_hand_pose"], f32).reshape(B, 45),
        np.asarray(inp["right_hand_pose"], f32).reshape(B, 45),
    ], axis=1)

    ep = np.asarray(inp["eyelid_params"], f32)
    aux = np.concatenate([
        np.asarray(inp["head_scale"], f32)[:, None],
        np.asarray(inp["left_hand_scale"], f32)[:, None],
        np.asarray(inp["right_hand_scale"], f32)[:, None],
        ep[:, 0:1], ep[:, 1:2],
        np.asarray(inp["head_pos_offset"], f32),
        np.asarray(inp["left_hand_pos_offset"], f32),
        np.asarray(inp["right_hand_pos_offset"], f32),
    ], axis=1)                                               # [128, 14]

    def beta_T(second):
        b = np.concatenate([np.asarray(inp["shape_params"], f32), second], 1)
        bt = np.zeros((384, B), f32)
        bt[:NL] = b.T
        bt[NL] = 1.0
        return bt.reshape(3, 128, B)

    betaT_s = beta_T(np.asarray(inp["body_exp"], f32))
    betaT_f = beta_T(np.asarray(inp["flame_exp"], f32))

    joff = np.asarray(inp["joints_offset"], f32)
    joffT = np.ascontiguousarray(joff.transpose(1, 2, 0)).reshape(55, 384)

    def mrel_T(par, nj):
        m = np.eye(nj, dtype=f32)
        for j in range(1, nj):
            if par[j] >= 0:
                m[j, par[j]] = -1.0
        return np.ascontiguousarray(m.T)

    betam = np.zeros((11, 1), f32)
    betam[:10, 0] = np.asarray(inp["mano_betas"], f32)[0]
    betam[10, 0] = 1.0

    def anc_T(par, nj):
        m = np.zeros((nj, nj), f32)
        for j in range(nj):
            a = j
            while a >= 0:
                m[j, a] = 1.0
                a = par[a]
        return np.ascontiguousarray(m.T)

    fpar = _fk_forest()
    rep = dict(aa=aa, aux=aux, betaT_s=betaT_s, betaT_f=betaT_f, joffT=joffT,
               mrelT_all=mrel_T(fpar, NJ_ALL), betam=betam,
               ancT_s=anc_T(SMPLX_PARENTS, 55), ancT_f=anc_T(FLAME_PARENTS, 5),
               ancT_m=anc_T(MANO_PARENTS, 16),
               ident=np.eye(128, dtype=f32))

    sd_m_flat = sd_m_np.reshape(VM * 3, 10)
    tmpl_m_flat = tmpl_m.reshape(VM * 3)

    in_maps = []
    vid_all = np.full((NCORES, ROWS), -1, np.int64)

    for c in range(NCORES):
        p_ids = plain_sv[pl_sp[c]:pl_sp[c + 1]]
        h_pos = np.arange(hd_sp[c], hd_sp[c + 1])
        h_sv, h_fv = head_sv[h_pos], head_ix[h_pos]
        l_pos = np.arange(hl_sp[c], hl_sp[c + 1])
        r_pos = l_pos                                         # same split for R
        l_sv, r_sv = s2l[l_pos], s2r[r_pos]

        vid = np.full(ROWS, -1, np.int64)
        vid[:len(p_ids)] = p_ids
        vid[N_PLAIN:N_PLAIN + len(h_sv)] = h_sv
        vid[N_PLAIN + N_HEAD:N_PLAIN + N_HEAD + len(l_sv)] = l_sv
        vid[N_PLAIN + N_HEAD + N_HL:N_PLAIN + N_HEAD + N_HL + len(r_sv)] = r_sv
        vid_all[c] = vid
        vok = vid >= 0
        vc = np.where(vok, vid, 0)

        # smplx shapedirs slab [NCH, 128(p=l), (c, lk, v)]
        sdp = np.zeros((ROWS, 3, 384), f32)
        sdp[:, :, :NL] = np.where(vok[:, None, None], sd_s_np[vc], 0.0)
        sdp[:, :, NL] = np.where(vok[:, None], tmpl_s[vc], 0.0)
        slab = sdp.reshape(NCH, 128, 3, 3, 128).transpose(0, 4, 2, 3, 1)
        sd_s = np.ascontiguousarray(slab).reshape(NCH, 128, 1152)

        colv = vc[:, None] * 3 + np.arange(3)[None, :]
        pdv = pd_s_np[:PD_S_K][:, colv]
        pdv = np.where(vok[None, :, None], pdv, 0.0)
        pdv = pdv.reshape(PD_S_K, NCH, 128, 3).transpose(1, 0, 3, 2)
        pd_s_a = np.ascontiguousarray(pdv[:, :128]).reshape(NCH, 128, 384)
        pd_s_b = np.ascontiguousarray(pdv[:, 128:]).reshape(NCH, PD_S_K - 128, 384)

        jr_s = np.ascontiguousarray(
            np.where(vok[:, None], jr_s_np[:, vc].T, 0.0).reshape(NCH, 128, 55))
        w_s = np.ascontiguousarray(
            np.where(vok[:, None], w_s_np[vc], 0.0)
            .reshape(NCH, 128, 55).transpose(0, 2, 1))

        # flame: 5 even + 3 gathered chunks
        fe = _pad_ids(np.arange(fe_sp[c], fe_sp[c + 1]), NFE_CH * 128)
        fg = _pad_ids(h_fv, N_HEAD)
        f_rows = np.concatenate([fe, fg])
        fok = f_rows >= 0
        fc = np.where(fok, f_rows, 0)
        sdfp = np.zeros((len(f_rows), 3, 384), f32)
        sdfp[:, :, :NL] = np.where(fok[:, None, None], sd_f_np[fc], 0.0)
        sdfp[:, :, NL] = np.where(fok[:, None], tmpl_f[fc], 0.0)
        slab = sdfp.reshape(-1, 128, 3, 3, 128).transpose(0, 4, 2, 3, 1)
        sd_f = np.ascontiguousarray(slab).reshape(-1, 128, 1152)

        jr_f = np.ascontiguousarray(
            np.where(fok[:NFE_CH * 128, None], jr_f_np[:, fc[:NFE_CH * 128]].T, 0.0)
            .reshape(NFE_CH, 128, 5))

        fgc, fgok = fc[NFE_CH * 128:], fok[NFE_CH * 128:]
        colf = fgc[:, None] * 3 + np.arange(3)[None, :]
        pdfv = pd_f_np[9:36][:, colf]
        pdfv = np.where(fgok[None, :, None], pdfv, 0.0)
        pdfv = pdfv.reshape(PD_F_K, 3, 128, 3).transpose(1, 0, 3, 2)
        pd_f = np.ascontiguousarray(pdfv).reshape(3, PD_F_K, 384)

        wre = np.zeros((3, 11, 128), f32)
        for k in range(3):
            rows, ok = fgc[k * 128:(k + 1) * 128], fgok[k * 128:(k + 1) * 128]
            wre[k, :5] = np.where(ok[None, :], w_f_np[rows].T, 0.0)
            wre[k, 5:8] = np.where(ok[None, :], re_np[rows].T, 0.0)
            wre[k, 8:11] = np.where(ok[None, :], le_np[rows].T, 0.0)

        # mano hands + J shard
        m_rows = np.stack([_pad_ids(l_pos, 128), _pad_ids(r_pos, 128)])
        mok = m_rows >= 0
        mc = np.where(mok, m_rows, 0)
        sd_m = np.zeros((2, 11, 384), f32)
        pd_m_a = np.zeros((2, 128, 384), f32)
        pd_m_b = np.zeros((2, PD_M_K - 128, 384), f32)
        w_m = np.zeros((2, 16, 128), f32)
        for h in range(2):
            sdm = np.where(mok[h][:, None, None], sd_m_np[mc[h]], 0.0)
            sd_m[h, :10] = sdm.transpose(2, 1, 0).reshape(10, 384)
            sd_m[h, 10] = np.where(mok[h][:, None], tmpl_m[mc[h]], 0.0).T.reshape(384)
            colm = mc[h][:, None] * 3 + np.arange(3)[None, :]
            pdm = pd_m_np[:, colm]
            pdm = np.where(mok[h][None, :, None], pdm, 0.0).transpose(0, 2, 1)
            pd_m_a[h] = pdm[:128].reshape(128, 384)
            pd_m_b[h] = pdm[128:].reshape(PD_M_K - 128, 384)
            w_m[h] = np.where(mok[h][None, :], w_m_np[mc[h]].T, 0.0)

        mj = _pad_ids(np.arange(mj_sp[c], mj_sp[c + 1]), NMJ_CH * 128)
        mjok = mj >= 0
        mjc = np.where(mjok, mj, 0)
        sd_mj = np.concatenate(
            [np.where(mjok[:, None], sd_m_flat[mjc], 0.0),
             np.where(mjok, tmpl_m_flat[mjc], 0.0)[:, None]], 1)
        sd_mj = np.ascontiguousarray(
            sd_mj.reshape(NMJ_CH, 128, 11).transpose(0, 2, 1))
        jreg_m = np.zeros((NMJ_CH * 128, 48), f32)
        vv, cc3 = mjc // 3, mjc % 3
        jj = np.arange(16)
        jreg_m[np.arange(NMJ_CH * 128)[:, None], jj[None, :] * 3 + cc3[:, None]] = \
            np.where(mjok[:, None], jr_m_np[:, vv].T, 0.0)
        jreg_m = jreg_m.reshape(NMJ_CH, 128, 48)

        m = dict(rep)
        m.update(sd_s=sd_s, pd_s_a=pd_s_a, pd_s_b=pd_s_b, jr_s=jr_s, w_s=w_s,
                 sd_f=sd_f, jr_f=jr_f, pd_f=pd_f, wre_f=wre,
                 sd_m=sd_m, pd_m_a=pd_m_a, pd_m_b=pd_m_b, w_m=w_m,
                 sd_mj=sd_mj, jreg_m=jreg_m)
        out = {}
        for k, v in m.items():
            if k in BF16_INPUTS:
                out[k] = np.ascontiguousarray(v.astype(BF16NP))
            else:
                out[k] = np.ascontiguousarray(v, f32)
        in_maps.append(out)

    return in_maps, vid_all


# ================================================================ device IR

def _build_nc():
    nc = bacc.Bacc("TRN2", target_bir_lowering=False, debug=False,
                   num_devices=NCORES)
    di = {}

    def din(name, shape):
        dt = BF16 if name in BF16_INPUTS else F32
        di[name] = nc.dram_tensor(name, list(shape), dt, kind="ExternalInput").ap()

    din("aa", (B, 165)); din("aux", (B, 14))
    din("betaT_s", (3, 128, 128)); din("betaT_f", (3, 128, 128))
    din("joffT", (55, 384))
    din("mrelT_all", (92, 92))
    din("ancT_s", (55, 55)); din("ancT_f", (5, 5)); din("ancT_m", (16, 16))
    din("betam", (11, 1)); din("ident", (128, 128))
    din("sd_s", (NCH, 128, 1152)); din("pd_s_a", (NCH, 128, 384))
    din("pd_s_b", (NCH, PD_S_K - 128, 384))
    din("jr_s", (NCH, 128, 55)); din("w_s", (NCH, 55, 128))
    din("sd_f", (8, 128, 1152)); din("jr_f", (NFE_CH, 128, 5))
    din("pd_f", (3, PD_F_K, 384)); din("wre_f", (3, 11, 128))
    din("sd_m", (2, 11, 384)); din("pd_m_a", (2, 128, 384))
    din("pd_m_b", (2, PD_M_K - 128, 384)); din("w_m", (2, 16, 128))
    din("sd_mj", (NMJ_CH, 11, 128)); din("jreg_m", (NMJ_CH, 128, 48))

    out_d = nc.dram_tensor("out", [ROWS, 384], F32, kind="ExternalOutput").ap()
    dbg_d = None
    if DEBUG:
        dbg_d = nc.dram_tensor("dbg", [128, 4096], F32, kind="ExternalOutput").ap()

    with tile.TileContext(nc) as tc:
        _emit(nc, tc, di, out_d, dbg_d)
    nc.compile()
    return nc


def _emit(nc, tc, di, out_d, dbg_d=None):
    levels = _fk_levels(_fk_forest())
    es = ExitStack()
    persist = es.enter_context(tc.tile_pool(name="persist", bufs=1))
    slabs = es.enter_context(tc.tile_pool(name="slabs", bufs=3))
    acc_cm = tc.tile_pool(name="acc", bufs=4, space="PSUM")
    acc = acc_cm.__enter__()
    acct_cm = tc.tile_pool(name="acct", bufs=2, space="PSUM")
    acct = acct_cm.__enter__()
    jpool_cm = tc.tile_pool(name="jpool", bufs=1, space="PSUM")
    jpool = jpool_cm.__enter__()
    dram = es.enter_context(tc.tile_pool(name="dram", bufs=1, space="DRAM"))

    V, S, G, T, DMA = nc.vector, nc.scalar, nc.gpsimd, nc.tensor, nc.sync

    def ptile(shape, name):
        return persist.tile(list(shape), F32, tag=name, name=name)

    # ---------------- constants / staged inputs --------------------------

    aa = ptile((B, 165), "aa"); DMA.dma_start(aa[:], di["aa"][:])
    aux = ptile((B, 14), "aux"); DMA.dma_start(aux[:], di["aux"][:])

    betaT_s = persist.tile([128, 384], BF16, tag="betaT_s", name="betaT_s")
    betaT_f = persist.tile([128, 384], BF16, tag="betaT_f", name="betaT_f")
    for lk in range(3):
        DMA.dma_start(betaT_s[:, lk * 128:(lk + 1) * 128], di["betaT_s"][lk])
    betam = persist.tile([11, 1], BF16, tag="betam", name="betam"); DMA.dma_start(betam[:], di["betam"][:])

    # preloaded small per-chunk tensors (one DMA each, persist in SBUF)
    jr_all = persist.tile([128, NCH * 55], BF16, tag="jr_all", name="jr_all")
    DMA.dma_start(jr_all[:].rearrange("p (n k) -> p n k", k=55),
                  di["jr_s"][:].rearrange("n p k -> p n k"))
    for lk in range(3):
        DMA.dma_start(betaT_f[:, lk * 128:(lk + 1) * 128], di["betaT_f"][lk])
    ident = ptile((128, 128), "ident")
    DMA.dma_start(ident[:], di["ident"][:])
    joffT = ptile((55, 384), "joffT"); DMA.dma_start(joffT[:], di["joffT"][:])
    mrelT_all = ptile((92, 92), "mrelT_all"); DMA.dma_start(mrelT_all[:], di["mrelT_all"][:])
    jrf_all = persist.tile([128, NFE_CH * 5], BF16, tag="jrf_all", name="jrf_all")
    DMA.dma_start(jrf_all[:].rearrange("p (n k) -> p n k", k=5),
                  di["jr_f"][:].rearrange("n p k -> p n k"))
    jrm_all = persist.tile([128, NMJ_CH * 48], BF16, tag="jrm_all", name="jrm_all")
    DMA.dma_start(jrm_all[:].rearrange("p (n k) -> p n k", k=48),
                  di["jreg_m"][:].rearrange("n p k -> p n k"))
    sdmj_all = persist.tile([11, NMJ_CH * 128], BF16, tag="sdmj_all", name="sdmj_all")
    DMA.dma_start(sdmj_all[:].rearrange("p (n k) -> p n k", k=128),
                  di["sd_mj"][:].rearrange("n p k -> p n k"))
    w_all = persist.tile([55, NCH * 128], BF16, tag="w_all", name="w_all")
    DMA.dma_start(w_all[:].rearrange("p (n k) -> p n k", k=128),
                  di["w_s"][:].rearrange("n p k -> p n k"))
    wre_all = persist.tile([11, 3 * 128], BF16, tag="wre_all", name="wre_all")
    DMA.dma_start(wre_all[:].rearrange("p (n k) -> p n k", k=128),
                  di["wre_f"][:].rearrange("n p k -> p n k"))
    wm_all = persist.tile([16, 2 * 128], BF16, tag="wm_all", name="wm_all")
    DMA.dma_start(wm_all[:].rearrange("p (n k) -> p n k", k=128),
                  di["w_m"][:].rearrange("n p k -> p n k"))
    sdm_all = persist.tile([11, 2 * 384], BF16, tag="sdm_all", name="sdm_all")
    DMA.dma_start(sdm_all[:].rearrange("p (n k) -> p n k", k=384),
                  di["sd_m"][:].rearrange("n p k -> p n k"))

    # early zero-fills (gpsimd queue, before any collective blocks it)
    jsb2 = ptile((21, 384), "jsb2")
    G.memset(jsb2[:], 0.0)
    rhs_f = persist.tile([11, 2304], BF16, tag="rhs_f", name="rhs_f")
    G.memset(rhs_f[:], 0.0)
    rot_all = ptile((B, NJ_ALL * 9), "rot_all")
    ra3 = rot_all[:].rearrange("p (j x) -> p j x", x=9)
    ra4 = rot_all[:].rearrange("p (j m n) -> p j m n", m=3, n=3)
    G.memset(rot_all[:], 0.0)
    G.memset(ra3[:, :, 0:9:4], 1.0)
    jall = ptile((96, 400), "jall")

    # ---------------- stage A: blend shapes + J partials ------------------
    jpt = jpool.tile([128, 512], F32, tag="jpsum", name="jpt")
    jps = jpt[:, 0:384]
    jpt2 = jpool.tile([128, 512], F32, tag="jpsum2", name="jpt2")
    jps_f = jpt2[:, 0:384]
    jps_m = jpt2[:, 384:385]

    vp_sbuf = [persist.tile([128, 384], BF16, tag=f"vp{i}", name=f"vp{i}")
               for i in range(NCH)]
    vpf_sbuf = [persist.tile([128, 384], BF16, tag=f"vpf{h}", name=f"vpf{h}")
                for h in range(3)]
    vpm_sbuf = [persist.tile([128, 384], BF16, tag=f"vpm{h}", name=f"vpm{h}")
                for h in range(2)]

    def sd_mms(pp, slab_t, betaT, last=True):
        for c3 in range(3):
            for lk in range(3):
                T.matmul(pp[:, c3 * 128:(c3 + 1) * 128],
                         slab_t[:, (c3 * 3 + lk) * 128:(c3 * 3 + lk + 1) * 128],
                         betaT[:, lk * 128:(lk + 1) * 128],
                         start=(lk == 0), stop=(lk == 2 and last))

    vsb = [persist.tile([128, 384], BF16, tag=f"vsb{i}", name=f"vsb{i}")
           for i in range(NCH)]
    vsf32 = {i: ptile((128, 384), f"vsf32{i}") for i in CH_PLAIN}

    # ---- A1: shape blend + J partials (everything the AllReduce needs) ----
    for i in range(NCH):
        sdt = slabs.tile((128, 1152), BF16, tag="sd_s")
        DMA.dma_start(sdt[:], di["sd_s"][i])
        pp = acc.tile([128, 384], F32, tag="vppsum", padded_shape=[128, 512])
        sd_mms(pp, sdt, betaT_s)
        S.copy(vsb[i][:], pp[:])
        if i in CH_PLAIN:
            V.tensor_copy(vsf32[i][:], pp[:])
        T.matmul(jps[0:55, :], jr_all[:, i * 55:(i + 1) * 55], vsb[i][:],
                 start=(i == 0), stop=(i == NCH - 1))

    # ---- AR1: smplx J AllReduce (launched before flame/mano A1) ----------
    jsb = ptile((55, 384), "jsb")
    S.copy(jsb[:], jps[0:55, :])
    ar_in1 = dram.tile([55, 384], F32, tag="ar_in1")
    ar_out1, _arf1 = tc.tile([55, 384], F32, space="DRAM", addr_space="Shared",
                             name="ar_out1")
    DMA.dma_start(ar_in1[:], jsb[:])
    G.collective_compute("AllReduce", ALU.add,
                         replica_groups=[list(range(NCORES))],
                         ins=[ar_in1[:].opt()], outs=[ar_out1[:].opt()])
    arr = ptile((55, 384), "arr")
    G.dma_start(arr[:], ar_out1[:])

    for k in range(NFE_CH):
        sdt = slabs.tile((128, 1152), BF16, tag="sd_f")
        DMA.dma_start(sdt[:], di["sd_f"][k])
        pp = acc.tile([128, 384], F32, tag="vppsum", padded_shape=[128, 512])
        sd_mms(pp, sdt, betaT_f)
        vsf = slabs.tile((128, 384), BF16, tag="vsf")
        S.copy(vsf[:], pp[:])
        T.matmul(jps_f[0:5, 0:384], jrf_all[:, k * 5:(k + 1) * 5], vsf[:],
                 start=(k == 0), stop=(k == NFE_CH - 1))

    for k in range(NMJ_CH):
        pp = acc.tile([128, 384], F32, tag="vppsum", padded_shape=[128, 512])
        T.matmul(pp[:, 0:1], sdmj_all[:, k * 128:(k + 1) * 128], betam[:],
                 start=True, stop=True)
        vsm = slabs.tile((128, 1), BF16, tag="vsmj")
        S.copy(vsm[:], pp[:, 0:1])
        T.matmul(jps_m[0:48, 0:1], jrm_all[:, k * 48:(k + 1) * 48], vsm[:],
                 start=(k == 0), stop=(k == NMJ_CH - 1))

    # ---- AR2: flame + mano J AllReduce -----------------------------------
    S.copy(jsb2[0:5, :], jps_f[0:5, 0:384])
    jsb_m = ptile((48, 1), "jsb_m")
    S.copy(jsb_m[:], jps_m[0:48, 0:1])
    jpool_cm.__exit__(None, None, None)
    ar_in2 = dram.tile([21, 384], F32, tag="ar_in2")
    ar_out2, _arf2 = tc.tile([21, 384], F32, space="DRAM", addr_space="Shared",
                             name="ar_out2")
    DMA.dma_start(ar_in2[:], jsb2[:])
    DMA.dma_start(ar_in2[5:21, 0:3], jsb_m[:])
    G.collective_compute("AllReduce", ALU.add,
                         replica_groups=[list(range(NCORES))],
                         ins=[ar_in2[:].opt()], outs=[ar_out2[:].opt()])
    G.dma_start(jall[55:60, 0:384], ar_out2[0:5, :])
    G.dma_start(jall[64:80, 384:387], ar_out2[5:21, 0:3])
    G.dma_start(jall[80:96, 384:387], ar_out2[5:21, 0:3])

    # ---- eyelid rows of rhs_f (early: only needs aux) --------------------
    epp = ptile((B, 2), "epp")
    V.tensor_mul(epp[:], aux[:, 3:5], aux[:, 0:1].broadcast_to([B, 2]))
    epT = persist.tile([2, 128], BF16, tag="epT", name="epT")
    ppe = acct.tile([128, 384], F32, tag="tpose", padded_shape=[128, 512])
    T.matmul(ppe[0:2, 0:128], epp[:, :], ident[:], is_transpose=True,
             start=True, stop=True)
    S.copy(epT[:], ppe[0:2, 0:128])
    for m3 in range(3):
        DMA.dma_start(rhs_f[5 + m3:6 + m3, (12 + m3) * 128:(13 + m3) * 128],
                      epT[1:2, :])
        DMA.dma_start(rhs_f[8 + m3:9 + m3, (15 + m3) * 128:(16 + m3) * 128],
                      epT[0:1, :])

    # ---------------- rodrigues (vector; overlaps A1 on tensor) -----------
    rot = ptile((B, NROT * 9), "rot")
    _rodrigues(nc, aa, rot, ptile)
    rot4 = rot[:].rearrange("p (j x) -> p j x", x=9)

    def pf_make(name, j0, n):
        t = ptile((B, n * 9), name)
        t9 = t[:].rearrange("p (j x) -> p j x", x=9)
        V.tensor_copy(t9, rot4[:, j0:j0 + n, :])
        V.tensor_scalar_add(t9[:, :, 0:9:4], t9[:, :, 0:9:4], -1.0)
        return t

    pf_s = pf_make("pf_s", 1, 21)
    pf_f = pf_make("pf_f", 22, 3)
    pf_m = [pf_make("pf_l", 25, 15), pf_make("pf_r", 40, 15)]

    def transpose_to(dst_ap, src_ap):
        pp = acct.tile([128, 384], F32, tag="tpose", padded_shape=[128, 512])
        k, n = src_ap.shape[0], src_ap.shape[1]
        T.matmul(pp[:n, :k], src_ap, ident[:k, :k], is_transpose=True,
                 start=True, stop=True)
        S.copy(dst_ap, pp[:n, :k])

    pfT_s_a = persist.tile([128, 128], BF16, tag="pfT_s_a", name="pfT_s_a")
    pfT_s_b = persist.tile([PD_S_K - 128, 128], BF16, tag="pfT_s_b", name="pfT_s_b")
    transpose_to(pfT_s_a[:], pf_s[:, 0:128])
    transpose_to(pfT_s_b[:], pf_s[:, 128:PD_S_K])
    pfT_f = persist.tile([PD_F_K, 128], BF16, tag="pfT_f", name="pfT_f")
    transpose_to(pfT_f[:], pf_f[:, :])
    pfT_m_a = [persist.tile([128, 128], BF16, tag="pfT_l_a", name="pfT_l_a"), persist.tile([128, 128], BF16, tag="pfT_r_a", name="pfT_r_a")]
    pfT_m_b = [persist.tile([PD_M_K - 128, 128], BF16, tag="pfT_l_b", name="pfT_l_b"),
               persist.tile([PD_M_K - 128, 128], BF16, tag="pfT_r_b", name="pfT_r_b")]
    for h in range(2):
        transpose_to(pfT_m_a[h][:], pf_m[h][:, 0:128])
        transpose_to(pfT_m_b[h][:], pf_m[h][:, 128:PD_M_K])

    # ---- world rotations (vector; independent of the AllReduce) ----------
    V.tensor_copy(ra3[:, 0:22, :], rot4[:, 0:22, :])
    V.tensor_copy(ra3[:, 57:60, :], rot4[:, 22:25, :])
    V.tensor_copy(ra3[:, 61:76, :], rot4[:, 25:40, :])
    V.tensor_copy(ra3[:, 77:92, :], rot4[:, 40:55, :])
    negid = persist.tile([55, 55], BF16, tag="negid", name="negid")
    V.tensor_scalar_mul(negid[:], ident[0:55, 0:55], -1.0)
    ones3 = persist.tile([3, 16], BF16, tag="ones3", name="ones3")
    V.memset(ones3[:], 1.0)

    Rw = ptile((B, NJ_ALL * 9), "Rw")
    Rw4 = Rw[:].rearrange("p (j m n) -> p j m n", m=3, n=3)
    fk_scr = ptile((B, 16 * 9), "fk_scr")

    def rw_mul(dst_sl, par_sl, loc_sl, n, par_bcast=False):
        dst = Rw4[:, dst_sl]
        par = Rw4[:, par_sl]
        if par_bcast:
            par = par.broadcast_to([B, n, 3, 3])
        loc = ra4[:, loc_sl]
        sc = fk_scr[:].rearrange("p (j m n) -> p j m n", m=3, n=3)[:, :n]
        for k in range(3):
            a_k = par[:, :, :, k:k + 1].broadcast_to([B, n, 3, 3])
            t_k = loc[:, :, k:k + 1, :].broadcast_to([B, n, 3, 3])
            if k == 0:
                V.tensor_mul(dst, a_k, t_k)
            else:
                V.tensor_mul(sc, a_k, t_k)
                V.tensor_add(dst, dst, sc)

    V.tensor_copy(Rw4[:, 0:1], ra4[:, 0:1])
    rw_mul(slice(1, 4), slice(0, 1), slice(1, 4), 3, par_bcast=True)
    rw_mul(slice(4, 7), slice(1, 4), slice(4, 7), 3)
    rw_mul(slice(7, 10), slice(4, 7), slice(7, 10), 3)
    rw_mul(slice(10, 13), slice(7, 10), slice(10, 13), 3)
    rw_mul(slice(13, 15), slice(9, 10), slice(13, 15), 2, par_bcast=True)
    rw_mul(slice(15, 18), slice(12, 15), slice(15, 18), 3)
    rw_mul(slice(18, 20), slice(16, 18), slice(18, 20), 2)
    V.tensor_copy(Rw4[:, 22:25], Rw4[:, 15:16].broadcast_to([B, 3, 3, 3]))
    rw_mul(slice(20, 22), slice(18, 20), slice(20, 22), 2)
    V.tensor_copy(Rw4[:, 25:40], Rw4[:, 20:21].broadcast_to([B, 15, 3, 3]))
    V.tensor_copy(Rw4[:, 40:55], Rw4[:, 21:22].broadcast_to([B, 15, 3, 3]))
    # flame roots/jaw/eyes + mano roots and level-1 (parents are identity)
    V.tensor_copy(Rw4[:, 55:61], ra4[:, 55:61])
    V.tensor_copy(Rw4[:, 76:77], ra4[:, 76:77])
    V.tensor_copy(Rw4[:, 61:74:3], ra4[:, 61:74:3])
    V.tensor_copy(Rw4[:, 77:90:3], ra4[:, 77:90:3])
    rw_mul(slice(62, 75, 3), slice(61, 74, 3), slice(62, 75, 3), 5)
    rw_mul(slice(78, 91, 3), slice(77, 90, 3), slice(78, 91, 3), 5)
    rw_mul(slice(63, 76, 3), slice(62, 75, 3), slice(63, 76, 3), 5)
    rw_mul(slice(79, 92, 3), slice(78, 91, 3), slice(79, 92, 3), 5)

    # ---- A2: posedirs + flame/mano v_posed (overlaps the AllReduce) ------
    for i in range(NCH):
        pda = slabs.tile((128, 384), BF16, tag="pd_s_a")
        pdb = slabs.tile((PD_S_K - 128, 384), BF16, tag="pd_s_b")
        DMA.dma_start(pda[:], di["pd_s_a"][i])
        DMA.dma_start(pdb[:], di["pd_s_b"][i])
        pq = acc.tile([128, 384], F32, tag="vppsum", padded_shape=[128, 512])
        for c3 in range(3):
            T.matmul(pq[:, c3 * 128:(c3 + 1) * 128],
                     pda[:, c3 * 128:(c3 + 1) * 128], pfT_s_a[:],
                     start=True, stop=False)
            T.matmul(pq[:, c3 * 128:(c3 + 1) * 128],
                     pdb[:, c3 * 128:(c3 + 1) * 128], pfT_s_b[:],
                     start=False, stop=True)
        if i in CH_PLAIN:
            V.tensor_add(vp_sbuf[i][:], vsf32[i][:], pq[:])
        else:
            S.copy(vp_sbuf[i][:], pq[:])

    for h in range(3):
        sdt = slabs.tile((128, 1152), BF16, tag="sd_f")
        DMA.dma_start(sdt[:], di["sd_f"][NFE_CH + h])
        pp = acc.tile([128, 384], F32, tag="vppsum", padded_shape=[128, 512])
        pdf = slabs.tile((PD_F_K, 384), BF16, tag="pd_f")
        DMA.dma_start(pdf[:], di["pd_f"][h])
        for c3 in range(3):
            for lk in range(3):
                T.matmul(pp[:, c3 * 128:(c3 + 1) * 128],
                         sdt[:, (c3 * 3 + lk) * 128:(c3 * 3 + lk + 1) * 128],
                         betaT_f[:, lk * 128:(lk + 1) * 128],
                         start=(lk == 0), stop=False)
            T.matmul(pp[:, c3 * 128:(c3 + 1) * 128],
                     pdf[:, c3 * 128:(c3 + 1) * 128], pfT_f[:],
                     start=False, stop=True)
        S.copy(vpf_sbuf[h][:], pp[:])

    for h in range(2):
        pps = acc.tile([128, 384], F32, tag="vppsum", padded_shape=[128, 512])
        for c3 in range(3):
            T.matmul(pps[:, c3:c3 + 1], sdm_all[:, h * 384 + c3 * 128:h * 384 + (c3 + 1) * 128],
                     betam[:], start=True, stop=True)
        vshm = ptile((128, 3), f"vshm{h}")
        S.copy(vshm[:], pps[:, 0:3])
        pda = slabs.tile((128, 384), BF16, tag="pd_m_a")
        pdb = slabs.tile((PD_M_K - 128, 384), BF16, tag="pd_m_b")
        DMA.dma_start(pda[:], di["pd_m_a"][h])
        DMA.dma_start(pdb[:], di["pd_m_b"][h])
        pq = acc.tile([128, 384], F32, tag="vppsum", padded_shape=[128, 512])
        for c3 in range(3):
            T.matmul(pq[:, c3 * 128:(c3 + 1) * 128],
                     pda[:, c3 * 128:(c3 + 1) * 128], pfT_m_a[h][:],
                     start=True, stop=False)
            T.matmul(pq[:, c3 * 128:(c3 + 1) * 128],
                     pdb[:, c3 * 128:(c3 + 1) * 128], pfT_m_b[h][:],
                     start=False, stop=True)
        vpm = vpm_sbuf[h]
        V.tensor_add(vpm[:].rearrange("p (c b) -> p c b", b=128),
                     pq[:].rearrange("p (c b) -> p c b", b=128),
                     vshm[:].unsqueeze(2).broadcast_to([128, 3, 128]))

    # ================= joints + A_rel assembly (post-AllReduce) ===========
    arr3 = arr  # smplx J sum from AR1
    V.tensor_add(jall[0:55, 0:384], arr3[:], joffT[:])
    # broadcast compact mano J into (c,b) layout on an aligned scratch tile,
    # then DMA into jall rows 60:92 (engine partition starts must be 32-aligned)
    jmtmp = ptile((32, 384), "jmtmp")
    V.tensor_copy(jmtmp[:].rearrange("p (c b) -> p c b", b=128),
                  jall[64:96, 384:387].unsqueeze(2).broadcast_to([32, 3, 128]))
    G.dma_start(jall[60:92, 0:384], jmtmp[:])

    # rel = mrel_all @ J (one fp32 matmul over the whole forest)
    ppr = acct.tile([128, 384], F32, tag="tpose", padded_shape=[128, 512])
    T.matmul(ppr[0:92, 0:384], mrelT_all[:], jall[0:92, 0:384],
             start=True, stop=True)
    rel_all = ptile((92, 384), "rel_all")
    S.copy(rel_all[:], ppr[0:92, 0:384])

    # batch-major J and rel:  jrb[:, 0:276] = J (c-major), [:, 280:556] = rel
    jrb = ptile((B, 560), "jrb")
    for c3 in range(3):
        ppj = acct.tile([128, 384], F32, tag="tpose", padded_shape=[128, 512])
        T.matmul(ppj[0:128, 0:92], jall[0:92, c3 * 128:(c3 + 1) * 128],
                 ident[0:92, 0:92], is_transpose=True, start=True, stop=True)
        T.matmul(ppj[0:128, 192:284], rel_all[:, c3 * 128:(c3 + 1) * 128],
                 ident[0:92, 0:92], is_transpose=True, start=True, stop=True)
        S.copy(jrb[:].rearrange("p (t x) -> p t x", x=280)[:, :, c3 * 92:(c3 + 1) * 92],
               ppj[:].rearrange("p (t x) -> p t x", x=192)[:, :, 0:92])

    jbv = jrb[:, 0:276].rearrange("p (c a) -> p c a", c=3)
    relv = jrb[:, 280:556].rearrange("p (c a) -> p c a", c=3)

    # q_a = R_a^T rel_a ; c_a = R^w_a q_a ; u_a = R^w_a J_a   (all joints)
    qv = ptile((B, 3 * NJ_ALL), "qv")
    cv = ptile((B, 3 * NJ_ALL), "cv")
    uv = ptile((B, 3 * NJ_ALL), "uv")
    scr3 = ptile((B, 3 * NJ_ALL), "scr3")
    q3 = qv[:].rearrange("p (a k) -> p a k", k=3)
    c3v = cv[:].rearrange("p (a k) -> p a k", k=3)
    u3 = uv[:].rearrange("p (a k) -> p a k", k=3)
    s3 = scr3[:].rearrange("p (a k) -> p a k", k=3)
    for m in range(3):
        rm = relv[:, m, :].unsqueeze(2).broadcast_to([B, NJ_ALL, 3])
        if m == 0:
            V.tensor_mul(q3, ra4[:, :, 0, :], rm)
        else:
            V.tensor_mul(s3, ra4[:, :, m, :], rm)
            V.tensor_add(q3, q3, s3)
    for k in range(3):
        qk = q3[:, :, k].unsqueeze(2).broadcast_to([B, NJ_ALL, 3])
        if k == 0:
            V.tensor_mul(c3v, Rw4[:, :, :, 0], qk)
        else:
            V.tensor_mul(s3, Rw4[:, :, :, k], qk)
            V.tensor_add(c3v, c3v, s3)
    for k in range(3):
        jk = jbv[:, k, :].unsqueeze(2).broadcast_to([B, NJ_ALL, 3])
        if k == 0:
            V.tensor_mul(u3, Rw4[:, :, :, 0], jk)
        else:
            V.tensor_mul(s3, Rw4[:, :, :, k], jk)
            V.tensor_add(u3, u3, s3)

    # ---- scale / mirror folding (batch-major) ----------------------------
    V.tensor_scalar_mul(Rw[:, 495:540], Rw[:, 495:540], aux[:, 0:1])
    V.tensor_scalar_mul(cv[:, 165:180], cv[:, 165:180], aux[:, 0:1])
    V.tensor_scalar_mul(uv[:, 165:180], uv[:, 165:180], aux[:, 0:1])
    negls = ptile((B, 1), "negls")
    V.tensor_scalar_mul(negls[:], aux[:, 1:2], -1.0)
    V.tensor_scalar_mul(Rw4[:, 60:76, 0, :], Rw4[:, 60:76, 0, :], negls[:, 0:1])
    V.tensor_scalar_mul(Rw4[:, 60:76, 1:3, :], Rw4[:, 60:76, 1:3, :], aux[:, 1:2])
    V.tensor_scalar_mul(c3v[:, 60:76, 0], c3v[:, 60:76, 0], negls[:, 0:1])
    V.tensor_scalar_mul(c3v[:, 60:76, 1:3], c3v[:, 60:76, 1:3], aux[:, 1:2])
    V.tensor_scalar_mul(u3[:, 60:76, 0], u3[:, 60:76, 0], negls[:, 0:1])
    V.tensor_scalar_mul(u3[:, 60:76, 1:3], u3[:, 60:76, 1:3], aux[:, 1:2])
    V.tensor_scalar_mul(Rw[:, 684:828], Rw[:, 684:828], aux[:, 2:3])
    V.tensor_scalar_mul(cv[:, 228:276], cv[:, 228:276], aux[:, 2:3])
    V.tensor_scalar_mul(uv[:, 228:276], uv[:, 228:276], aux[:, 2:3])

    # ---- per-batch bias vectors (head / left / right) --------------------
    bias9 = ptile((B, 9), "bias9")
    hm = ptile((B, 6), "hm")
    hl = ptile((B, 3), "hl")
    hr = ptile((B, 3), "hr")
    V.tensor_add(hm[:, 0:3], jbv[:, :, 23], jbv[:, :, 24])
    V.tensor_add(hm[:, 3:6], jbv[:, :, 58], jbv[:, :, 59])
    V.tensor_sub(bias9[:, 0:3], hm[:, 0:3], hm[:, 3:6])
    V.tensor_scalar_mul(bias9[:, 0:3], bias9[:, 0:3], 0.5)
    V.tensor_add(bias9[:, 0:3], bias9[:, 0:3], aux[:, 5:8])
    V.tensor_sub(hl[:], aux[:, 8:11], jbv[:, :, 60])
    V.tensor_sub(bias9[:, 3:4], jbv[:, 0:1, 20], hl[:, 0:1])
    V.tensor_add(bias9[:, 4:6], jbv[:, 1:3, 20], hl[:, 1:3])
    V.tensor_sub(hr[:], aux[:, 11:14], jbv[:, :, 60])
    V.tensor_add(bias9[:, 6:9], jbv[:, :, 21], hr[:])

    # bsT per group -> [1, 384] bias rhs rows (DMA reshapes [3,128]->[1,384])
    rbias = []
    for g in range(3):
        ppb = acct.tile([128, 384], F32, tag="tpose", padded_shape=[128, 512])
        T.matmul(ppb[0:3, 0:128], bias9[:, g * 3:(g + 1) * 3], ident[:],
                 is_transpose=True, start=True, stop=True)
        bst = persist.tile([3, 128], BF16, tag=f"bsT{g}", name=f"bsT{g}")
        S.copy(bst[:], ppb[0:3, 0:128])
        rb = persist.tile([1, 384], BF16, tag=f"rbias{g}", name=f"rbias{g}")
        G.dma_start(rb[:], bst[:])
        rbias.append(rb)

    # ---- rhs assembly ----------------------------------------------------
    rhs_s = persist.tile([55, 1536], BF16, tag="rhs_s", name="rhs_s")
    rhs_m = [persist.tile([16, 1536], BF16, tag="rhs_l", name="rhs_l"),
             persist.tile([16, 1536], BF16, tag="rhs_r", name="rhs_r")]
    groups = [(0, 55, rhs_s, di["ancT_s"], None),
              (55, 5, rhs_f, di["ancT_f"], 0),
              (60, 16, rhs_m[0], di["ancT_m"], 1),
              (76, 16, rhs_m[1], di["ancT_m"], 2)]
    ancT_t = {}
    for nm in ("ancT_s", "ancT_f", "ancT_m"):
        n = di[nm].shape[0]
        t = persist.tile([n, n], BF16, tag=nm, name=nm + "_t")
        DMA.dma_start(t[:], di[nm][:])
        ancT_t[nm] = t

    for (a0, ng, rhs_t, anc_d, bias_g) in groups:
        anc_t = ancT_t["ancT_s" if ng == 55 else ("ancT_f" if ng == 5 else "ancT_m")]
        # cT/uT via strided transposes:  cuT[:, 0:384]=c^T, [:, 384:768]=u^T
        cuT = persist.tile([ng, 768], BF16, tag=f"cuT{a0}", name=f"cuT{a0}")
        for m in range(3):
            ppc = acct.tile([128, 384], F32, tag="tpose", padded_shape=[128, 512])
            T.matmul(ppc[0:ng, 0:128],
                     cv[:, 3 * a0 + m:3 * (a0 + ng - 1) + m + 1:3],
                     ident[:], is_transpose=True, start=True, stop=True)
            T.matmul(ppc[0:ng, 192:320],
                     uv[:, 3 * a0 + m:3 * (a0 + ng - 1) + m + 1:3],
                     ident[:], is_transpose=True, start=True, stop=True)
            S.copy(cuT[0:ng].rearrange("p (t x) -> p t x", x=384)[:, :, m * 128:(m + 1) * 128],
                   ppc[0:ng].rearrange("p (t x) -> p t x", x=192)[:, :, 0:128])
        # translations: Anc @ c - u (+ bias) -> rhs cols 1152:1536
        ppt = acct.tile([128, 384], F32, tag="tpose", padded_shape=[128, 512])
        T.matmul(ppt[0:ng, 0:384], anc_t[:], cuT[0:ng, 0:384],
                 start=True, stop=False)
        T.matmul(ppt[0:ng, 0:384], negid[0:ng, 0:ng], cuT[0:ng, 384:768],
                 start=False, stop=(bias_g is None))
        if bias_g is not None:
            T.matmul(ppt[0:ng, 0:384], ones3[0:1, 0:ng], rbias[bias_g][:],
                     start=False, stop=True)
        S.copy(rhs_t[0:ng, 1152:1536], ppt[0:ng, 0:384])
        # rotations: R^w columns -> rhs cols n*384 + m*128
        for n4 in range(3):
            ppn = acct.tile([128, 384], F32, tag="tpose", padded_shape=[128, 512])
            for m in range(3):
                T.matmul(ppn[0:ng, m * 128:(m + 1) * 128],
                         Rw[:, 9 * a0 + m * 3 + n4:9 * (a0 + ng - 1) + m * 3 + n4 + 1:9],
                         ident[:], is_transpose=True, start=True, stop=True)
            S.copy(rhs_t[0:ng, n4 * 384:(n4 + 1) * 384], ppn[0:ng, 0:384])

    acct_cm.__exit__(None, None, None)
    acc_cm.__exit__(None, None, None)
    big_cm = tc.tile_pool(name="big", bufs=2, space="PSUM")
    big = big_cm.__enter__()

    # ---------------- skinning per chunk (bf16 DVE fast-path) -------------
    def t_apply(dst_ap, tp_bf, x_sbuf, scratch):
        """dst = sum_{n<3} T'[n]*x_n + T'[3]; bf16 SBUF operands."""
        d3 = dst_ap.rearrange("p (m b) -> p m b", b=128)
        x3 = x_sbuf[:].rearrange("p (c b) -> p c b", b=128)
        tp = tp_bf[:].rearrange("p (n m b) -> p n m b", m=3, b=128)
        sc = scratch[:].rearrange("p (n m b) -> p n m b", m=3, b=128)
        V.tensor_mul(sc[:, 0:3], tp[:, 0:3],
                     x3[:].unsqueeze(2).broadcast_to([128, 3, 3, 128]))
        V.tensor_add(sc[:, 0], sc[:, 0], sc[:, 1])
        V.tensor_add(sc[:, 0], sc[:, 0], tp[:, 3])
        V.tensor_add(d3, sc[:, 0], sc[:, 2])

    scr_t = [persist.tile([128, 1152], BF16, tag=f"scr{i}", name=f"scr{i}")
             for i in range(4)]

    for i in range(NCH):
        if CH_HEAD0 <= i < CH_HEAD0 + 3:
            h = i - CH_HEAD0
            hv = slabs.tile((128, 384), BF16, tag="hv", bufs=2, name="hv")
            wt = wre_all[:, h * 128:(h + 1) * 128]
            tp1 = big.tile([128, 1536], F32, tag="bigp")
            for g, w in ((0, 512), (1, 512), (2, 128)):
                T.matmul(tp1[:, g * 512:g * 512 + w], wt,
                         rhs_f[:, g * 512:g * 512 + w], start=True, stop=True)
            tp2 = big.tile([128, 1536], F32, tag="bigp")
            for g, w in ((0, 512), (1, 512), (2, 128)):
                T.matmul(tp2[:, g * 512:g * 512 + w], wt,
                         rhs_f[:, 1152 + g * 512:1152 + g * 512 + w],
                         start=True, stop=True)
            tb1 = slabs.tile((128, 1152), BF16, tag="tpb1", bufs=2, name="tb1")
            S.copy(tb1[:], tp1[:, 0:1152])
            tb2 = slabs.tile((128, 1152), BF16, tag="tpb2", bufs=2, name="tb2")
            S.copy(tb2[:], tp2[:, 0:1152])
            d3 = hv[:].rearrange("p (m b) -> p m b", b=128)
            x3 = vpf_sbuf[h][:].rearrange("p (c b) -> p c b", b=128)
            t1 = tb1[:].rearrange("p (n m b) -> p n m b", m=3, b=128)
            t2 = tb2[:].rearrange("p (n m b) -> p n m b", m=3, b=128)
            sc = scr_t[i % 4][:].rearrange("p (n m b) -> p n m b", m=3, b=128)
            V.tensor_mul(sc[:, 0:3], t1[:, 0:3],
                         x3[:].unsqueeze(2).broadcast_to([128, 3, 3, 128]))
            V.tensor_add(sc[:, 0], sc[:, 0], sc[:, 1])
            V.tensor_add(sc[:, 0], sc[:, 0], sc[:, 2])
            V.tensor_add(sc[:, 1], t2[:, 0], t2[:, 1])
            V.tensor_add(sc[:, 1], sc[:, 1], t2[:, 2])
            V.tensor_add(d3, sc[:, 0], sc[:, 1])
            V.tensor_add(vp_sbuf[i][:], vp_sbuf[i][:], hv[:])
        elif i in (CH_HL, CH_HR):
            h = i - CH_HL
            hv = slabs.tile((128, 384), BF16, tag="hv", bufs=2, name="hv")
            wt = wm_all[:, h * 128:(h + 1) * 128]
            tpm = big.tile([128, 1536], F32, tag="bigp")
            for g in range(3):
                T.matmul(tpm[:, g * 512:(g + 1) * 512], wt,
                         rhs_m[h][:, g * 512:(g + 1) * 512], start=True, stop=True)
            tbm = slabs.tile((128, 1536), BF16, tag="tpbm", bufs=2, name="tbm")
            S.copy(tbm[:], tpm[:])
            t_apply(hv[:], tbm, vpm_sbuf[h], scr_t[i % 4])
            V.tensor_add(vp_sbuf[i][:], vp_sbuf[i][:], hv[:])

        wt = w_all[:, i * 128:(i + 1) * 128]
        tps = big.tile([128, 1536], F32, tag="bigp")
        for g in range(3):
            T.matmul(tps[:, g * 512:(g + 1) * 512], wt,
                     rhs_s[:, g * 512:(g + 1) * 512], start=True, stop=True)
        tbs = slabs.tile((128, 1536), BF16, tag="tpbs", bufs=3, name="tbs")
        S.copy(tbs[:], tps[:])
        ot = slabs.tile((128, 384), F32, tag="outt", bufs=3, name="ot")
        t_apply(ot[:], tbs, vp_sbuf[i], scr_t[i % 4])
        DMA.dma_start(out_d[i * 128:(i + 1) * 128, :], ot[:])

    big_cm.__exit__(None, None, None)
    es.close()


def _rodrigues(nc, aa, rot, ptile):
    V, S = nc.vector, nc.scalar
    J = NROT
    aa3 = aa[:].rearrange("p (j k) -> p j k", k=3)
    sq = ptile((B, J), "rg_sq")
    tmp = ptile((B, J), "rg_tmp")
    V.tensor_mul(sq[:], aa3[:, :, 0], aa3[:, :, 0])
    V.tensor_mul(tmp[:], aa3[:, :, 1], aa3[:, :, 1])
    V.tensor_add(sq[:], sq[:], tmp[:])
    V.tensor_mul(tmp[:], aa3[:, :, 2], aa3[:, :, 2])
    V.tensor_add(sq[:], sq[:], tmp[:])
    eps_t = ptile((B, 1), "rg_eps")
    nc.gpsimd.memset(eps_t[:], 1e-8)
    hpi_t = ptile((B, 1), "rg_hpi")
    nc.gpsimd.memset(hpi_t[:], float(np.pi / 2))
    zero_t = ptile((B, 1), "rg_zero")
    nc.gpsimd.memset(zero_t[:], 0.0)
    ang = ptile((B, J), "rg_ang")
    S.activation(ang[:], sq[:], AF.Sqrt, bias=eps_t[:])
    inv = ptile((B, J), "rg_inv")
    V.reciprocal(inv[:], ang[:])
    sn = ptile((B, J), "rg_sin")
    co = ptile((B, J), "rg_cos")
    S.activation(sn[:], ang[:], AF.Sin, bias=zero_t[:])
    S.activation(co[:], ang[:], AF.Sin, bias=hpi_t[:])
    nv = ptile((B, 3 * J), "rg_n")
    n3 = nv[:].rearrange("p (j k) -> p j k", k=3)
    V.tensor_mul(n3, aa3, inv[:].unsqueeze(2).broadcast_to([B, J, 3]))
    u = ptile((B, J), "rg_u")
    V.tensor_scalar(u[:], co[:], -1.0, 1.0, ALU.mult, ALU.add)
    un = ptile((B, 3 * J), "rg_un")
    un3 = un[:].rearrange("p (j k) -> p j k", k=3)
    V.tensor_mul(un3, n3, u[:].unsqueeze(2).broadcast_to([B, J, 3]))
    q = ptile((B, 3 * J), "rg_q")
    q3 = q[:].rearrange("p (j k) -> p j k", k=3)
    V.tensor_mul(q3, un3, n3)
    d = ptile((B, J), "rg_d")
    V.tensor_add(d[:], q3[:, :, 0], q3[:, :, 1])
    V.tensor_add(d[:], d[:], q3[:, :, 2])
    dd = ptile((B, J), "rg_dd")
    V.tensor_scalar(dd[:], d[:], -1.0, 1.0, ALU.mult, ALU.add)
    snv = ptile((B, 3 * J), "rg_snv")
    s3 = snv[:].rearrange("p (j k) -> p j k", k=3)
    V.tensor_mul(s3, n3, sn[:].unsqueeze(2).broadcast_to([B, J, 3]))
    r4 = rot[:].rearrange("p (j m n) -> p j m n", m=3, n=3)
    for m in range(3):
        V.tensor_add(r4[:, :, m, m], q3[:, :, m], dd[:])
    p = ptile((B, J), "rg_p")
    V.tensor_mul(p[:], un3[:, :, 0], n3[:, :, 1])
    V.tensor_sub(r4[:, :, 0, 1], p[:], s3[:, :, 2])
    V.tensor_add(r4[:, :, 1, 0], p[:], s3[:, :, 2])
    V.tensor_mul(p[:], un3[:, :, 0], n3[:, :, 2])
    V.tensor_add(r4[:, :, 0, 2], p[:], s3[:, :, 1])
    V.tensor_sub(r4[:, :, 2, 0], p[:], s3[:, :, 1])
    V.tensor_mul(p[:], un3[:, :, 1], n3[:, :, 2])
    V.tensor_sub(r4[:, :, 1, 2], p[:], s3[:, :, 0])
    V.tensor_add(r4[:, :, 2, 1], p[:], s3[:, :, 0])


# ================================================================ entry

_CACHED = {}
DEBUG = False


def _get_nc():
    if "nc" not in _CACHED:
        _CACHED["nc"] = _build_nc()
    return _CACHED["nc"]


PROFILE = False
TRACE_DIR = None


def kernel(**inputs):
    in_maps, vid_all = _host_prep(inputs)
    nc = _get_nc()
    kw = {}
    if PROFILE and TRACE_DIR:
        kw["tmpdir"] = TRACE_DIR
    res = run_bass_kernel_spmd(nc, in_maps, core_ids=list(range(NCORES)),
                               trace=PROFILE, **kw)
    _CACHED["last_res"] = res
    out = np.zeros((B, VS, 3), np.float32)
    for c in range(NCORES):
        o = np.asarray(res.results[c]["out"]).reshape(ROWS, 3, B)
        vok = vid_all[c] >= 0
        out[:, vid_all[c][vok], :] = o[vok].transpose(2, 0, 1)
    return out

